# revision 8
# baseline (speedup 1.0000x reference)
"""Capsule-network kernel for 8x TRN2 NeuronCores (data-parallel over batch).

Reference computation (see problem):
  prim = primary_input.reshape(B, 8, 1024)
  prev = zeros(B, 4096)
  for col in 0..3:
    # layer0: inp = [prim_t, x_t, col] (1537) @ W0 -> relu -> flat -> roll(-128)
    # layer1: inp = [x_t, col] (513) @ W1 -> relu -> flat -> roll(+128)
  out = prev @ W_out + b_out

Kernel strategy (per core, batch shard Bc=512):
  - Feature-major tiles [128 features, Bc batch]; ROLL=128 == partition
    count so rolls are free tile re-indexings.
  - The scalar `col` concat input folds into per-col biases (host).
  - P = prim @ W0[0:1024] is col-invariant: computed once, kept in SBUF
    as bf16 (scale 512), re-injected into each col's layer0 psum via an
    identity-bf16 matmul (213ns/tile) instead of a DVE add (658ns/tile).
  - col 0 layer0 has x=0 -> C = relu(P + b0): no x matmuls.
  - ALL GEMMs run as fp8(e4m3) DoubleRow perf mode: 0.5 cycles/row
    (2x the fp32r/bf16 rate).  Accuracy is recovered with a hi+lo
    split: for activations x and weights W,
       hi = e4m3(s*v), lo = e4m3(s*v - hi)   (s: 16 for x, 32 for W)
    and each GEMM computes x_hi@W_hi + x_hi@W_lo + x_lo@W_hi (the lo*lo
    term is dropped).  All three terms share one psum at scale 512.
    End-to-end rel err vs the fp32 reference: ~3.8e-3 (gate 2e-2).
  - DoubleRow contracts 256 rows/instruction: activation tiles are
    stored as PAIR tiles [128, 2, 512] (two adjacent 128-feature tiles
    in one contraction chunk).  Because both layers consume feature
    tiles in (odd, even) adjacent pairs after the rolls, a single pair
    layout (pair p = tiles 2p+1, 2p+2 mod 32) serves layer0, layer1 and
    the final GEMM.
  - Drains per tile: ACT relu_b = Relu(psum + 512*b) -> bf16;
    DVE hi8 = (relu_b * 1/32) max 0 -> fp8; GPSIMD lo8 = (relu_b *
    1/32) - hi8 -> fp8.  Three engines, each under the tensor budget.
  - Matmul sweeps keep (weights) stationary across the 8-row t sweep.
    Within layer0/layer1 fo-sweeps, the chunk whose input tiles drain
    earliest is emitted first to chase the previous layer's drains.
  - ~12 dummy matmuls ramp the PE p-state during the initial DMA wait.
"""

import numpy as np

# ---- problem constants (hardcoded; kernel.py must be self-contained) ----
B_FULL = 4096
D_IN = 8192
T = 8            # NUM_TALL
NW = 4           # NUM_WIDE
F = 512          # feature size per capsule row
ROLL = 128
N_CORES = 8
BC = B_FULL // N_CORES   # per-core batch = 512
S = (F * T) // 128       # state feature tiles = 32
FO = F // 128            # output feature tiles per row-layer = 4
NPAIR = S // 2           # activation pair tiles = 16
PC = (D_IN // T) // 256  # prim DoubleRow chunks per capsule row = 4
XC = F // 256            # x DoubleRow chunks per layer = 2
N_OUT = 10
N_OUT_PAD = 16          # DoubleRow stationary half width (10 padded to 16)
N_WARM = 12              # dummy p-state warmup matmuls

SX = 16.0                # activation fp8 scale
SW = 32.0                # weight fp8 scale
SP = SX * SW             # psum scale = 512

_CACHE = {}


def _pair_of(j):
    """State tile j -> (pair index, half). Pair p = tiles (2p+1, 2p+2 % 32)."""
    return ((j - 1) % S) // 2, 1 - (j % 2)


def _build_program():
    """Build (and cache) the single-core Bass program. Same program runs
    SPMD on all 8 cores with different batch shards."""
    if "nc" in _CACHE:
        return _CACHE["nc"], _CACHE["names"]

    from contextlib import ExitStack

    import concourse.tile as tile
    from concourse import bacc, mybir

    f32 = mybir.dt.float32
    f8 = mybir.dt.float8e4
    bf16 = mybir.dt.bfloat16
    AF = mybir.ActivationFunctionType
    ALU = mybir.AluOpType
    DR = mybir.MatmulPerfMode.DoubleRow

    nc = bacc.Bacc("TRN2", target_bir_lowering=False, debug=False,
                   num_devices=N_CORES)

    prim_hi_d = nc.dram_tensor("prim_hi", [T * PC * 128, 2, BC], f8,
                               kind="ExternalInput").ap()
    prim_lo_d = nc.dram_tensor("prim_lo", [T * PC * 128, 2, BC], f8,
                               kind="ExternalInput").ap()
    w0p_hi_d = nc.dram_tensor("w0p_hi", [PC * 128, 2, F], f8,
                              kind="ExternalInput").ap()
    w0p_lo_d = nc.dram_tensor("w0p_lo", [PC * 128, 2, F], f8,
                              kind="ExternalInput").ap()
    w0x_hi_d = nc.dram_tensor("w0x_hi", [XC * 128, 2, F], f8,
                              kind="ExternalInput").ap()
    w0x_lo_d = nc.dram_tensor("w0x_lo", [XC * 128, 2, F], f8,
                              kind="ExternalInput").ap()
    w1x_hi_d = nc.dram_tensor("w1x_hi", [XC * 128, 2, F], f8,
                              kind="ExternalInput").ap()
    w1x_lo_d = nc.dram_tensor("w1x_lo", [XC * 128, 2, F], f8,
                              kind="ExternalInput").ap()
    wout_hi_d = nc.dram_tensor("wout_hi", [128, NPAIR, 2, N_OUT_PAD], f8,
                               kind="ExternalInput").ap()
    wout_lo_d = nc.dram_tensor("wout_lo", [128, NPAIR, 2, N_OUT_PAD], f8,
                               kind="ExternalInput").ap()
    ident_d = nc.dram_tensor("ident", [128, 128], bf16,
                             kind="ExternalInput").ap()
    bias0_d = nc.dram_tensor("bias0", [128, NW * FO], f32,
                             kind="ExternalInput").ap()
    bias1_d = nc.dram_tensor("bias1", [128, NW * FO], f32,
                             kind="ExternalInput").ap()
    bout_d = nc.dram_tensor("bout", [N_OUT, 1], f32, kind="ExternalInput").ap()
    out_d = nc.dram_tensor("out", [N_OUT, BC], f32, kind="ExternalOutput").ap()

    with tile.TileContext(nc) as tc, ExitStack() as ctx:
        const = ctx.enter_context(tc.tile_pool(name="const", bufs=1))
        state = ctx.enter_context(tc.tile_pool(name="state", bufs=1))
        cpool = ctx.enter_context(tc.tile_pool(name="cpool", bufs=44))
        rpool = ctx.enter_context(tc.tile_pool(name="rpool", bufs=10))
        prim_pool = ctx.enter_context(tc.tile_pool(name="primp", bufs=34))
        ppool = ctx.enter_context(tc.tile_pool(name="psum", bufs=8, space="PSUM"))

        # ---- constants ----
        w0p_h = [const.tile([128, 2, F], f8, name=f"w0ph{c}", tag=f"w0ph{c}")
                 for c in range(PC)]
        w0p_l = [const.tile([128, 2, F], f8, name=f"w0pl{c}", tag=f"w0pl{c}")
                 for c in range(PC)]
        w0x_h = [const.tile([128, 2, F], f8, name=f"w0xh{q}", tag=f"w0xh{q}")
                 for q in range(XC)]
        w0x_l = [const.tile([128, 2, F], f8, name=f"w0xl{q}", tag=f"w0xl{q}")
                 for q in range(XC)]
        w1x_h = [const.tile([128, 2, F], f8, name=f"w1xh{q}", tag=f"w1xh{q}")
                 for q in range(XC)]
        w1x_l = [const.tile([128, 2, F], f8, name=f"w1xl{q}", tag=f"w1xl{q}")
                 for q in range(XC)]
        wout_h = const.tile([128, NPAIR, 2, N_OUT_PAD], f8, name="wouth", tag="wouth")
        wout_l = const.tile([128, NPAIR, 2, N_OUT_PAD], f8, name="woutl", tag="woutl")
        ident_sb = const.tile([128, 128], bf16, name="ident_sb", tag="ident")
        bias0_sb = const.tile([128, NW * FO], f32, name="bias0_sb", tag="bias0")
        bias1_sb = const.tile([128, NW * FO], f32, name="bias1_sb", tag="bias1")
        bout_sb = const.tile([N_OUT, 1], f32, name="bout_sb", tag="bout")
        warm_sb = const.tile([128, 128], f32, name="warm_sb", tag="warm")

        # ---- persistent state ----
        # A pairs (layer1 outputs), P tiles (prim projection, scale 512)
        Ah = [state.tile([128, 2, BC], f8, name=f"ah{p}", tag=f"Ah{p}")
              for p in range(NPAIR)]
        Al = [state.tile([128, 2, BC], f8, name=f"al{p}", tag=f"Al{p}")
              for p in range(NPAIR)]
        P = [state.tile([128, BC], bf16, name=f"state_p{i}", tag=f"P{i}")
             for i in range(S)]

        # ---- p-state warmup: dummy matmuls on a memset tile ----
        nc.vector.memset(warm_sb[:], 0.0)
        for i in range(N_WARM):
            ps = ppool.tile([128, BC], f32, name=f"warm{i}", tag="mm")
            nc.tensor.matmul(ps[0:128, 0:128], warm_sb[:], warm_sb[:],
                             start=True, stop=True)

        def load_deferred_consts(gi):
            # late-needed constants ride the mostly-idle gpsimd (SWDGE)
            # queue so the sync/scalar queues keep streaming prim.
            if gi == 0:
                nc.scalar.dma_start(bias0_sb[:], bias0_d[:, :])
                nc.gpsimd.dma_start(bias1_sb[:], bias1_d[:, :])
                nc.gpsimd.dma_start(bout_sb[:], bout_d[:, :])
                for q in range(XC):
                    nc.gpsimd.dma_start(w1x_h[q][:], w1x_hi_d[q * 128:(q + 1) * 128])
                    nc.gpsimd.dma_start(w1x_l[q][:], w1x_lo_d[q * 128:(q + 1) * 128])
            elif gi == 1:
                for q in range(XC):
                    nc.gpsimd.dma_start(w0x_h[q][:], w0x_hi_d[q * 128:(q + 1) * 128])
                    nc.gpsimd.dma_start(w0x_l[q][:], w0x_lo_d[q * 128:(q + 1) * 128])
            elif gi == 2:
                nc.gpsimd.dma_start(ident_sb[:], ident_d[:, :])
            elif gi == 3:
                nc.gpsimd.dma_start(wout_h[:], wout_hi_d[:, :, :, :])
                nc.gpsimd.dma_start(wout_l[:], wout_lo_d[:, :, :, :])

        # col-current layer0 output pairs (cpool ring tiles)
        Ch = [None] * NPAIR
        Cl = [None] * NPAIR

        TERMS = (("h", "h"), ("l", "h"), ("h", "l"))  # (w side, x side)

        def drain(ps, j, bias_ap, dsth, dstl, with_p):
            """psum(scale 512) -> [P[j] copy,] rl(bf16, scale 16), hi8, lo8.

            rl = Relu(psum/32 + 16b) so hi8 is a plain cast and lo8 a
            plain subtract (cheapest DVE forms; all-SBUF operands keep
            the 2x_2p door open)."""
            p, h = _pair_of(j)
            if with_p:
                nc.scalar.copy(P[j][:], ps[:])
            rl = rpool.tile([128, BC], bf16, name=f"rl{j}", tag="rl")
            nc.scalar.activation(rl[:], ps[:], AF.Relu, bias=bias_ap,
                                 scale=1.0 / SW)
            nc.vector.tensor_copy(dsth[p][:, h, :], rl[:])
            nc.vector.tensor_tensor(dstl[p][:, h, :], rl[:],
                                    dsth[p][:, h, :], ALU.subtract)

        # ==== phase 1: P = prim @ W0p (col-invariant), fused col0-L0 ====
        # Rows grouped [0], (1,2), (3,4), (5,6), [7]: within a group the
        # stationary tile is reused across rows.
        groups = [(0,), (1, 2), (3, 4), (5, 6), (7,)]

        for p in range(NPAIR):
            Ch[p] = cpool.tile([128, 2, BC], f8, name=f"c0h_{p}", tag="C")
            Cl[p] = cpool.tile([128, 2, BC], f8, name=f"c0l_{p}", tag="C")

        def prim_dma(t, c):
            # one pair-tile (hi + lo) per (row, chunk): 1KB lines.
            # All prim rides the sync queue: the ACT engine (scalar
            # queue) is saturated by phase-1 relu + P-copy drains.
            g = t * PC + c
            th = prim_pool.tile([128, 2, BC], f8, name=f"prh_{g}", tag="prim")
            tl = prim_pool.tile([128, 2, BC], f8, name=f"prl_{g}", tag="prim")
            nc.sync.dma_start(th[:], prim_hi_d[g * 128:(g + 1) * 128])
            nc.sync.dma_start(tl[:], prim_lo_d[g * 128:(g + 1) * 128])
            return {"h": th, "l": tl}

        for gi, grp in enumerate(groups):
            pss = {}
            for t in grp:
                for fo in range(FO):
                    pss[(t, fo)] = ppool.tile([128, BC], f32,
                                              name=f"ps_p1_{t}_{fo}", tag="mm")
            pt = {}
            if gi == 0:
                # interleave w0p with row 0's tiles on scalar so each
                # chunk's stationary+moving pair lands together
                for c in range(PC):
                    nc.scalar.dma_start(w0p_h[c][:],
                                        w0p_hi_d[c * 128:(c + 1) * 128])
                    nc.scalar.dma_start(w0p_l[c][:],
                                        w0p_lo_d[c * 128:(c + 1) * 128])
                    pt[(0, c)] = prim_dma(0, c)
            else:
                for c in range(PC):
                    for t in grp:
                        pt[(t, c)] = prim_dma(t, c)
            for c in range(PC):
                for fo in range(FO):
                    for ti, (ws, xs) in enumerate(TERMS):
                        w = (w0p_h if ws == "h" else w0p_l)[c]
                        for t in grp:
                            nc.tensor.matmul(
                                pss[(t, fo)][:],
                                w[:, :, fo * 128:(fo + 1) * 128],
                                pt[(t, c)][xs][:, :, :],
                                start=(c == 0 and ti == 0),
                                stop=(c == PC - 1 and ti == len(TERMS) - 1),
                                perf_mode=DR)
            load_deferred_consts(gi)
            for t in grp:
                for fo in range(FO):
                    j = t * FO + fo
                    drain(pss[(t, fo)], j, bias0_sb[:, fo:fo + 1],
                          Ch, Cl, with_p=True)

        # ==== layer emitters: stationary runs of 8 across the t sweep ====
        def layer1_col(c):
            # A[4t+fo] = relu(W1x.T C + b1c); C chunks = pairs (2t, 2t+1)
            for fo in range(FO):
                pss = [ppool.tile([128, BC], f32, name=f"ps1_{c}_{fo}_{t}",
                                  tag="mm") for t in range(T)]
                n = 0
                for q in range(XC):  # pair 2t first: its tiles drain earlier
                    for ws, xs in TERMS:
                        w = (w1x_h if ws == "h" else w1x_l)[q]
                        for t in range(T):
                            rhs = (Ch if xs == "h" else Cl)[(2 * t + q) % NPAIR]
                            nc.tensor.matmul(
                                pss[t][:], w[:, :, fo * 128:(fo + 1) * 128],
                                rhs[:, :, :],
                                start=(n == 0), stop=(n == 2 * len(TERMS) - 1),
                                perf_mode=DR)
                        n += 1
                b1ap = bias1_sb[:, c * FO + fo:c * FO + fo + 1]
                for t in range(T):
                    drain(pss[t], t * FO + fo, b1ap, Ah, Al, with_p=False)

        def layer0_col(c):
            # C[4t+fo] = relu(W0x.T x + P + b0c); x chunks = pairs
            # (2t-1, 2t).  P is injected via an identity-bf16 matmul.
            # Rotation: t sweep starts at 1 (pair 2t-1 at t=0 is pair 15,
            # holding tile 31 -- the previous layer1's very last drain).
            rows = [(1 + i) % T for i in range(T)]
            for p in range(NPAIR):
                Ch[p] = cpool.tile([128, 2, BC], f8, name=f"c{c}h_{p}", tag="C")
                Cl[p] = cpool.tile([128, 2, BC], f8, name=f"c{c}l_{p}", tag="C")
            for fo in range(FO):
                pss = {t: ppool.tile([128, BC], f32, name=f"ps0_{c}_{fo}_{t}",
                                     tag="mm") for t in rows}
                for t in rows:
                    nc.tensor.matmul(pss[t][:], ident_sb[:, :],
                                     P[t * FO + fo][:], start=True, stop=False)
                n = 0
                for q in (1, 0):  # pair 2t first (drains earlier), 2t-1 last
                    for ws, xs in TERMS:
                        w = (w0x_h if ws == "h" else w0x_l)[q]
                        for t in rows:
                            rhs = (Ah if xs == "h" else Al)[(2 * t + q - 1) % NPAIR]
                            nc.tensor.matmul(
                                pss[t][:], w[:, :, fo * 128:(fo + 1) * 128],
                                rhs[:, :, :],
                                start=False, stop=(n == 2 * len(TERMS) - 1),
                                perf_mode=DR)
                        n += 1
                b0ap = bias0_sb[:, c * FO + fo:c * FO + fo + 1]
                for t in rows:
                    drain(pss[t], t * FO + fo, b0ap, Ch, Cl, with_p=False)

        # ==== col 0 layer1, then cols 1..3 ====
        layer1_col(0)
        for c in range(1, NW):
            layer0_col(c)
            layer1_col(c)

        # ---- final: out = prev @ W_out + b_out (DoubleRow over A pairs) --
        # Even pairs (tiles fo1/fo2 of col3-L1) are ready first; odd pairs
        # (fo3/fo0) chase the last sweep's drains.
        psf_full = ppool.tile([128, BC], f32, name="psf", tag="mm")
        psf = psf_full[0:N_OUT_PAD, :]
        psf10 = psf_full[0:N_OUT, :]
        n = 0
        for plist in (range(0, NPAIR, 2), range(1, NPAIR, 2)):
            for p in plist:
                for ws, xs in TERMS:
                    w = (wout_h if ws == "h" else wout_l)[:, p, :, :]
                    rhs = (Ah if xs == "h" else Al)[p]
                    nc.tensor.matmul(psf[:], w, rhs[:, :, :],
                                     start=(n == 0),
                                     stop=(n == NPAIR * len(TERMS) - 1),
                                     perf_mode=DR)
                    n += 1
        out_sb = rpool.tile([N_OUT, BC], f32, name="out_sb", tag="rl")
        # out = (psum + 512*b_out) * (1/512)
        nc.vector.tensor_scalar(out_sb[:], psf10[:], bout_sb[:], 1.0 / SP,
                                ALU.add, ALU.mult)
        nc.sync.dma_start(out_d[:, :], out_sb[:])

    nc.compile()

    names = {}
    _CACHE["nc"] = nc
    _CACHE["names"] = names
    return nc, names


def _split8(v, s):
    """hi = e4m3(s*v), lo = e4m3(s*v - hi); float8 ml_dtypes arrays."""
    import ml_dtypes
    F8 = ml_dtypes.float8_e4m3
    sv = s * v
    hi = sv.astype(F8)
    lo = (sv - hi.astype(np.float32)).astype(F8)
    return hi, lo


def _pack_pairs_w(w):
    """[K, F] -> [K/2, 2, F] DoubleRow pair chunks (K = n*256)."""
    K, Fdim = w.shape
    nch = K // 256
    return np.ascontiguousarray(
        w.reshape(nch, 2, 128, Fdim).transpose(0, 2, 1, 3)
        .reshape(nch * 128, 2, Fdim))


def _make_in_maps(primary_input, W0, b0, W1, b1, W_out, b_out):
    """Host-side sharding + fp8 hi/lo split + DoubleRow pair packing."""
    primary_input = np.ascontiguousarray(primary_input, dtype=np.float32)
    W0 = np.asarray(W0, dtype=np.float32)
    b0 = np.asarray(b0, dtype=np.float32)
    W1 = np.asarray(W1, dtype=np.float32)
    b1 = np.asarray(b1, dtype=np.float32)
    W_out = np.asarray(W_out, dtype=np.float32)
    b_out = np.asarray(b_out, dtype=np.float32)

    import ml_dtypes
    ps = D_IN // T  # 1024
    w0p_hi, w0p_lo = _split8(W0[:ps], SW)
    w0x_hi, w0x_lo = _split8(W0[ps:ps + F], SW)
    w1x_hi, w1x_lo = _split8(W1[:F], SW)
    w0_last = W0[ps + F]
    w1_last = W1[F]

    # wout pairs: pair p holds feature tiles (2p+1, 2p+2 % 32)
    # prev = roll(A_flat, +128): prev k-tile k == A tile (k-1)%32, so the
    # Wout chunk paired with A pair p (tiles 2p+1, 2p+2) holds Wout
    # tiles (2p+2, 2p+3).
    wo = W_out.reshape(S, 128, N_OUT).transpose(0, 2, 1)  # [S, 10, 128]
    wop = np.zeros((128, NPAIR, 2, N_OUT_PAD), np.float32)
    for p in range(NPAIR):
        wop[:, p, 0, :N_OUT] = wo[(2 * p + 2) % S].T
        wop[:, p, 1, :N_OUT] = wo[(2 * p + 3) % S].T
    wout_hi, wout_lo = _split8(wop, SW)

    # layer biases at scale 16 (the relu drain applies scale 1/32 to the
    # scale-512 psum); the final bias stays at psum scale 512.
    bias0 = np.concatenate(
        [(SX * (b0 + c * w0_last)).reshape(FO, 128).T for c in range(NW)],
        axis=1)
    bias1 = np.concatenate(
        [(SX * (b1 + c * w1_last)).reshape(FO, 128).T for c in range(NW)],
        axis=1)
    bias0 = np.ascontiguousarray(bias0, dtype=np.float32)   # [128, 16]
    bias1 = np.ascontiguousarray(bias1, dtype=np.float32)   # [128, 16]
    bout = np.ascontiguousarray((SP * b_out).reshape(N_OUT, 1),
                                dtype=np.float32)
    ident = np.ascontiguousarray(np.eye(128, dtype=np.float32)
                                 .astype(ml_dtypes.bfloat16))

    shared = dict(
        w0p_hi=_pack_pairs_w(w0p_hi), w0p_lo=_pack_pairs_w(w0p_lo),
        w0x_hi=_pack_pairs_w(w0x_hi), w0x_lo=_pack_pairs_w(w0x_lo),
        w1x_hi=_pack_pairs_w(w1x_hi), w1x_lo=_pack_pairs_w(w1x_lo),
        wout_hi=wout_hi, wout_lo=wout_lo,
        ident=ident, bias0=bias0, bias1=bias1, bout=bout)

    def pack_prim(v):
        # [8192, BC] -> [T*PC*128, 2, BC]: row (t*4+c)*128+p, half h
        # holds prim_t row t*1024 + (2c+h)*128 + p.
        return np.ascontiguousarray(
            v.reshape(T, PC, 2, 128, BC).transpose(0, 1, 3, 2, 4)
            .reshape(T * PC * 128, 2, BC))

    in_maps = []
    for core in range(N_CORES):
        shard = primary_input[core * BC:(core + 1) * BC]          # [512, 8192]
        prim_t = np.ascontiguousarray(shard.T)                    # [8192, 512]
        hi, lo = _split8(prim_t, SX)
        m = {"prim_hi": pack_prim(hi), "prim_lo": pack_prim(lo)}
        m.update(shared)
        in_maps.append(m)
    return in_maps


def _install_ntff_hook():
    """Provide antenv.axon_hooks (absent in this image) backed by ctypes
    calls into libaxon_pjrt.so, so run_bass_kernel_spmd(trace=True) can
    capture NTFF profiles. Mirrors trn_agent_boot.trn_boot."""
    import contextlib
    import ctypes
    import sys
    import types

    if "antenv.axon_hooks" in sys.modules:
        return
    so_path = "/opt/axon/libaxon_pjrt.so"
    lib = ctypes.CDLL(so_path)
    lib.axon_start_nrt_profile.argtypes = [ctypes.POINTER(ctypes.c_int64),
                                           ctypes.c_size_t]
    lib.axon_start_nrt_profile.restype = ctypes.c_int64
    lib.axon_stop_nrt_profile.argtypes = [ctypes.c_char_p]
    lib.axon_stop_nrt_profile.restype = ctypes.c_int64

    @contextlib.contextmanager
    def _hook(output_dir, device_ids):
        import jax
        jax.devices()
        if device_ids:
            ids = (ctypes.c_int64 * len(device_ids))(*device_ids)
            rc = lib.axon_start_nrt_profile(ids, len(device_ids))
        else:
            rc = lib.axon_start_nrt_profile(None, 0)
        if rc != 0:
            raise RuntimeError(f"axon_start_nrt_profile rc={rc}")
        try:
            yield
        finally:
            n = lib.axon_stop_nrt_profile(str(output_dir).encode())
            print(f"profile: {n} file(s) written to {output_dir}",
                  file=sys.stderr)

    mod = types.ModuleType("antenv.axon_hooks")
    mod.get_axon_ntff_profile_hook = lambda: _hook
    mod.set_axon_ntff_profile_hook = lambda h: None
    sys.modules["antenv.axon_hooks"] = mod
    import antenv
    antenv.axon_hooks = mod


def kernel(primary_input, W0, b0, W1, b1, W_out, b_out, _trace=False,
           _trace_cores=None):
    from concourse import bass_utils

    if _trace:
        _install_ntff_hook()

    nc, _ = _build_program()
    in_maps = _make_in_maps(primary_input, W0, b0, W1, b1, W_out, b_out)
    res = bass_utils.run_bass_kernel_spmd(
        nc, in_maps, core_ids=list(range(N_CORES)),
        trace=_trace, trace_cores=_trace_cores)
    out = np.empty((B_FULL, N_OUT), dtype=np.float32)
    for core in range(N_CORES):
        out[core * BC:(core + 1) * BC] = res.results[core]["out"].T
    if _trace:
        kernel._last_results = res
    return out


# revision 10
# speedup vs baseline: 1.8532x; 1.8532x over previous
"""Capsule-network kernel for 8x TRN2 NeuronCores (data-parallel over batch).

Reference computation (see problem):
  prim = primary_input.reshape(B, 8, 1024)
  prev = zeros(B, 4096)
  for col in 0..3:
    # layer0: inp = [prim_t, x_t, col] (1537) @ W0 -> relu -> flat -> roll(-128)
    # layer1: inp = [x_t, col] (513) @ W1 -> relu -> flat -> roll(+128)
  out = prev @ W_out + b_out

Kernel strategy (per core, batch shard Bc=512):
  - Everything on-chip is FEATURE-MAJOR: tiles are [128 features, Bc batch].
    ROLL=128 == partition count, so rolls are free tile re-indexings.
  - The scalar `col` concat input contributes col*W[last_row] to the
    pre-activation -> folded into per-col biases (computed on host).
  - P = prim @ W0[0:1024] is col-invariant -> computed once (phase 1),
    kept in SBUF as bf16, added during the layer0 drain each col.
  - col 0 layer0 has x=0 -> out = relu(P + b0): no matmuls at all.
  - Matmuls run as bf16 (same 1 col/cycle PE rate as fp32r, but
    the 2-byte LDWEIGHTS hides fully: measured cadence ~216ns vs
    ~227ns for fp32r).  Activations/weights bf16, psum fp32.
  - HW measurement: an fp32r matmul whose stationary weights differ from
    the previous matmul costs ~252ns; same-weights runs cost ~226.7ns.
    So layers are swept (fo, k) outer / row t inner: 8 consecutive
    matmuls share one weight tile (one sweep = 8 psum banks).
  - ~16 dummy matmuls at t=0 (on a memset tile) ramp the PE out of its
    low p-state during the initial DMA wait.
"""

import numpy as np

# ---- problem constants (hardcoded; kernel.py must be self-contained) ----
B_FULL = 4096
D_IN = 8192
T = 8            # NUM_TALL
NW = 4           # NUM_WIDE
F = 512          # feature size per capsule row
ROLL = 128
N_CORES = 8
BC = B_FULL // N_CORES   # per-core batch = 512
S = (F * T) // 128       # state feature tiles = 32
KP = (D_IN // T) // 128  # prim k-tiles per capsule row = 8
KX = F // 128            # x k-tiles = 4
FO = F // 128            # output feature tiles per row-layer = 4
N_OUT = 10
N_WARM = 12              # dummy p-state warmup matmuls

_CACHE = {}


def _build_program():
    """Build (and cache) the single-core Bass program. Same program runs
    SPMD on all 8 cores with different batch shards."""
    if "nc" in _CACHE:
        return _CACHE["nc"], _CACHE["names"]

    from contextlib import ExitStack

    import concourse.tile as tile
    from concourse import bacc, mybir

    f32 = mybir.dt.float32
    f32r = mybir.dt.float32r
    bf16 = mybir.dt.bfloat16
    AF = mybir.ActivationFunctionType
    ADD = mybir.AluOpType.add

    nc = bacc.Bacc("TRN2", target_bir_lowering=False, debug=False,
                   num_devices=N_CORES)

    # prim + w0p travel and multiply as bf16: phase-1 is the only
    # DMA-heavy span (16.8MB of prim in fp32 saturates the 360 GB/s DMA
    # system during the cold start); halving the bytes costs ~26ns/matmul
    # of bf16 LDWEIGHTS overhead on the 256 P-matmuls but removes all
    # DMA-starvation stalls.
    prim_d = nc.dram_tensor("prim_t", [D_IN, BC], bf16, kind="ExternalInput").ap()
    w0p_d = nc.dram_tensor("w0p", [KP * 128, F], bf16, kind="ExternalInput").ap()
    w0x_d = nc.dram_tensor("w0x", [F, F], bf16, kind="ExternalInput").ap()
    w1x_d = nc.dram_tensor("w1x", [F, F], bf16, kind="ExternalInput").ap()
    wout_d = nc.dram_tensor("wout_packed", [128, S * N_OUT], bf16,
                            kind="ExternalInput").ap()
    bias0_d = nc.dram_tensor("bias0", [128, NW * FO], f32, kind="ExternalInput").ap()
    bias1_d = nc.dram_tensor("bias1", [128, NW * FO], f32, kind="ExternalInput").ap()
    bout_d = nc.dram_tensor("bout", [N_OUT, 1], f32, kind="ExternalInput").ap()
    out_d = nc.dram_tensor("out", [N_OUT, BC], f32, kind="ExternalOutput").ap()

    with tile.TileContext(nc) as tc, ExitStack() as ctx:
        const = ctx.enter_context(tc.tile_pool(name="const", bufs=1))
        state = ctx.enter_context(tc.tile_pool(name="state", bufs=1))
        cpool = ctx.enter_context(tc.tile_pool(name="cpool", bufs=33))
        prim_pool = ctx.enter_context(tc.tile_pool(name="primp", bufs=12))
        ppool = ctx.enter_context(tc.tile_pool(name="psum", bufs=8, space="PSUM"))

        # ---- constants ----
        w0p_sb = [const.tile([128, F], bf16, name=f"w0p{k}", tag=f"w0p{k}")
                  for k in range(KP)]
        w0x_sb = [const.tile([128, F], bf16, name=f"w0x{k}", tag=f"w0x{k}")
                  for k in range(KX)]
        w1x_sb = [const.tile([128, F], bf16, name=f"w1x{k}", tag=f"w1x{k}")
                  for k in range(KX)]
        wout_sb = const.tile([128, S * N_OUT], bf16, name="wout_sb", tag="wout")
        bias0_sb = const.tile([128, NW * FO], f32, name="bias0_sb", tag="bias0")
        bias1_sb = const.tile([128, NW * FO], f32, name="bias1_sb", tag="bias1")
        bout_sb = const.tile([N_OUT, 1], f32, name="bout_sb", tag="bout")
        warm_sb = const.tile([128, 128], f32, name="warm_sb", tag="warm")

        # ---- persistent state ----
        A = [state.tile([128, BC], bf16, name=f"state_a{i}", tag=f"A{i}")
             for i in range(S)]
        P = [state.tile([128, BC], bf16, name=f"state_p{i}", tag=f"P{i}")
             for i in range(S)]

        # ---- p-state warmup: dummy matmuls on a memset tile ----
        # (fp32 runs at 4 cycles/row so a 128-wide moving dim gives
        # ~213-790ns per dummy across the ramp)
        nc.vector.memset(warm_sb[:], 0.0)
        for i in range(N_WARM):
            ps = ppool.tile([128, BC], f32, name=f"warm{i}", tag="mm")
            nc.tensor.matmul(ps[0:128, 0:128], warm_sb[:], warm_sb[:],
                             start=True, stop=True)

        def load_deferred_consts(gi):
            # late-needed constants ride the idle gpsimd (SWDGE) queue so
            # the sync/scalar queues keep streaming prim.  w1x must be
            # resident by ~15us (first col0-layer1 chunk), so it goes out
            # at gi=0 -- the gpsimd queue has nothing else and issues it
            # at t~1us.
            if gi == 0:
                nc.scalar.dma_start(bias0_sb[:], bias0_d[:, :])
                nc.gpsimd.dma_start(bias1_sb[:], bias1_d[:, :])
                nc.gpsimd.dma_start(bout_sb[:], bout_d[:, :])
                for k in range(KX):
                    nc.gpsimd.dma_start(w1x_sb[k][:], w1x_d[k * 128:(k + 1) * 128, :])
            elif gi == 1:
                for k in range(KX):
                    nc.gpsimd.dma_start(w0x_sb[k][:], w0x_d[k * 128:(k + 1) * 128, :])
            elif gi == 3:
                nc.gpsimd.dma_start(wout_sb[:], wout_d[:, :])

        C = [None] * S  # col-current layer0 outputs (cpool ring tiles)

        # ==== phase 1: P[j] = prim @ W0p (col-invariant), fused col0-L0 ====
        # Rows grouped [0], (1,2), (3,4), (5,6), [7]: within a group the
        # (k, fo) weight tile is reused across rows (run-of-2).
        # Drains: P copy (DVE, bf16) + col0-L0 C = relu(P+b0) (ACT).
        groups = [(0,), (1, 2), (3, 4), (5, 6), (7,)]

        def prim_dma(t, k):
            # one k-tile per DMA, full-width: the DMA system is
            # descriptor-line-rate limited, so 1KB lines (full bf16 rows)
            # move twice the bytes per line vs split halves.  Row 0 and
            # odd rows ride the scalar queue (it starts issuing ~5us
            # before sync, which carries the TileContext preamble).
            g = t * KP + k
            tile_ = prim_pool.tile([128, BC], bf16, name=f"prim_{g}",
                                   tag="prim")
            q = nc.scalar if (t == 0 or t % 2 == 1) else nc.sync
            q.dma_start(tile_[:], prim_d[g * 128:(g + 1) * 128, :])
            return tile_

        def layer1_chunk(c, rows):
            # layer1 for a subset of rows (weight run-of-len(rows)).
            # A[4t+fo] = relu(W1x.T C + b1c);  C k-tile = C[(4t+k+1)%S]
            for fo in range(FO):
                pss = {t: ppool.tile([128, BC], f32, name=f"ps1_{c}_{fo}_{t}",
                                     tag="mm") for t in rows}
                for k in range(KX):
                    w_ap = w1x_sb[k][:, fo * 128:(fo + 1) * 128]
                    for t in rows:
                        nc.tensor.matmul(
                            pss[t][:], w_ap, C[(t * FO + k + 1) % S][:],
                            start=(k == 0), stop=(k == KX - 1))
                b1ap = bias1_sb[:, c * FO + fo:c * FO + fo + 1]
                for t in rows:
                    j = t * FO + fo
                    if t % 2 == 0:
                        nc.scalar.activation(A[j][:], pss[t][:], AF.Relu,
                                             bias=b1ap)
                    else:
                        # relu(psum + bias) on DVE: (psum add bias) max 0
                        nc.vector.tensor_scalar(A[j][:], pss[t][:], b1ap, 0.0,
                                                ADD, mybir.AluOpType.max)

        for gi, grp in enumerate(groups):
            pss = {}
            for t in grp:
                for fo in range(FO):
                    pss[(t, fo)] = ppool.tile([128, BC], f32,
                                              name=f"ps_p1_{t}_{fo}", tag="mm")
            pt = {}
            if gi == 0:
                # interleave w0p with row 0's tiles on scalar so the
                # k-th matmul's pair (w0p[k], prim(0,k)) lands together
                for k in range(KP):
                    nc.scalar.dma_start(w0p_sb[k][:],
                                        w0p_d[k * 128:(k + 1) * 128, :])
                    pt[(0, k)] = prim_dma(0, k)
            else:
                for k in range(KP):
                    for t in grp:
                        pt[(t, k)] = prim_dma(t, k)
            for k in range(KP):
                for fo in range(FO):
                    for t in grp:
                        nc.tensor.matmul(
                            pss[(t, fo)][:],
                            w0p_sb[k][:, fo * 128:(fo + 1) * 128],
                            pt[(t, k)][:],
                            start=(k == 0), stop=(k == KP - 1))
            load_deferred_consts(gi)
            for t in grp:
                for fo in range(FO):
                    j = t * FO + fo
                    nc.vector.tensor_copy(P[j][:], pss[(t, fo)][:])
                    ct = cpool.tile([128, BC], bf16, name=f"c0_{j}", tag="C")
                    nc.scalar.activation(ct[:], pss[(t, fo)][:], AF.Relu,
                                         bias=bias0_sb[:, fo:fo + 1])
                    C[j] = ct
        # col-0 layer1 with full run-of-8 weight reuse (bf16 phase-1 DMA
        # leaves enough bandwidth slack that no absorber work is needed)
        layer1_chunk(0, tuple(range(T)))

        # ==== layer emitters: (fo, k) outer, t inner -> weight run-of-8 ====
        def layer0_col(c):
            # C[4t+fo] = relu(W0x.T x + P + b0c);  x k-tile = A[(4t+k-1)%S]
            # The t sweep starts at t=1: the k=0 input A[4t-1] is a fo3
            # tile of the previous col's layer1, and t=0 needs A[31] --
            # the very LAST drain of that col.  Rotating gives each A
            # one extra sweep-step of drain slack.
            rows = [(1 + i) % T for i in range(T)]
            for fo in range(FO):
                pss = {t: ppool.tile([128, BC], f32, name=f"ps0_{c}_{fo}_{t}",
                                     tag="mm") for t in rows}
                for k in range(KX):
                    w_ap = w0x_sb[k][:, fo * 128:(fo + 1) * 128]
                    for t in rows:
                        nc.tensor.matmul(
                            pss[t][:], w_ap, A[(t * FO + k - 1) % S][:],
                            start=(k == 0), stop=(k == KX - 1))
                b0ap = bias0_sb[:, c * FO + fo:c * FO + fo + 1]
                for t in rows:
                    j = t * FO + fo
                    ct = cpool.tile([128, BC], bf16, name=f"c{c}_{j}", tag="C")
                    # ct = (psum + bias0_c) + P  on DVE, then relu on ACT
                    nc.vector.scalar_tensor_tensor(
                        ct[:], pss[t][:], b0ap, P[j][:], ADD, ADD)
                    nc.scalar.activation(ct[:], ct[:], AF.Relu)
                    C[j] = ct

        def layer1_col(c):
            # A[4t+fo] = relu(W1x.T C + b1c);  C k-tile = C[(4t+k+1)%S]
            for fo in range(FO):
                pss = [ppool.tile([128, BC], f32, name=f"ps1_{c}_{fo}_{t}",
                                  tag="mm") for t in range(T)]
                for k in range(KX):
                    w_ap = w1x_sb[k][:, fo * 128:(fo + 1) * 128]
                    for t in range(T):
                        nc.tensor.matmul(
                            pss[t][:], w_ap, C[(t * FO + k + 1) % S][:],
                            start=(k == 0), stop=(k == KX - 1))
                b1ap = bias1_sb[:, c * FO + fo:c * FO + fo + 1]
                for t in range(T):
                    j = t * FO + fo
                    if t % 2 == 0:
                        nc.scalar.activation(A[j][:], pss[t][:], AF.Relu,
                                             bias=b1ap)
                    else:
                        # relu(psum + bias) on DVE: (psum add bias) max 0
                        nc.vector.tensor_scalar(A[j][:], pss[t][:], b1ap, 0.0,
                                                ADD, mybir.AluOpType.max)

        # ==== cols 1..3 (col-0 layer1 was interleaved into phase 1) ====
        for c in range(1, NW):
            layer0_col(c)
            layer1_col(c)

        # ---- final: out = prev @ W_out + b_out;  prev[k] = A[(k-1) % S] ----
        psf_full = ppool.tile([128, BC], f32, name="psf", tag="mm")
        psf = psf_full[0:N_OUT, :]
        # emit in col-3's A-write order (sweep fo, then t) so the
        # accumulation chain chases the layer1 drains
        n = 0
        for fo in range(FO):
            for t in range(T):
                k = (t * FO + fo + 1) % S
                nc.tensor.matmul(
                    psf[:],
                    wout_sb[:, k * N_OUT:(k + 1) * N_OUT],
                    A[(k - 1) % S][:],
                    start=(n == 0), stop=(n == S - 1))
                n += 1
        out_sb = cpool.tile([N_OUT, BC], f32, name="out_sb", tag="C")
        nc.scalar.activation(out_sb[:], psf[:], AF.Identity, bias=bout_sb[:])
        nc.sync.dma_start(out_d[:, :], out_sb[:])

    nc.compile()

    names = dict(prim="prim_t", w0p="w0p", w0x="w0x", w1x="w1x",
                 wout="wout_packed", bias0="bias0", bias1="bias1",
                 bout="bout", out="out")
    _CACHE["nc"] = nc
    _CACHE["names"] = names
    return nc, names


def _make_in_maps(primary_input, W0, b0, W1, b1, W_out, b_out):
    """Host-side sharding + layout prep (all cheap numpy except the
    feature-major transpose of the batch shards)."""
    primary_input = np.ascontiguousarray(primary_input, dtype=np.float32)
    W0 = np.asarray(W0, dtype=np.float32)
    b0 = np.asarray(b0, dtype=np.float32)
    W1 = np.asarray(W1, dtype=np.float32)
    b1 = np.asarray(b1, dtype=np.float32)
    W_out = np.asarray(W_out, dtype=np.float32)
    b_out = np.asarray(b_out, dtype=np.float32)

    import ml_dtypes
    ps = D_IN // T  # 1024
    w0p = np.ascontiguousarray(W0[:ps].astype(ml_dtypes.bfloat16))  # [1024, 512]
    w0x = np.ascontiguousarray(W0[ps:ps + F].astype(ml_dtypes.bfloat16))
    w0_last = W0[ps + F]                             # [512]
    w1x = np.ascontiguousarray(W1[:F].astype(ml_dtypes.bfloat16))
    w1_last = W1[F]                                  # [512]

    bias0 = np.concatenate(
        [(b0 + c * w0_last).reshape(FO, 128).T for c in range(NW)], axis=1)
    bias1 = np.concatenate(
        [(b1 + c * w1_last).reshape(FO, 128).T for c in range(NW)], axis=1)
    bias0 = np.ascontiguousarray(bias0, dtype=np.float32)   # [128, 16]
    bias1 = np.ascontiguousarray(bias1, dtype=np.float32)   # [128, 16]

    # wout_packed[p, k*10+o] = W_out[128k+p, o]
    wout_packed = np.ascontiguousarray(
        W_out.reshape(S, 128, N_OUT).transpose(1, 0, 2).reshape(128, S * N_OUT)
        .astype(ml_dtypes.bfloat16))
    bout = np.ascontiguousarray(b_out.reshape(N_OUT, 1))

    shared = dict(w0p=w0p, w0x=w0x, w1x=w1x, wout_packed=wout_packed,
                  bias0=bias0, bias1=bias1, bout=bout)
    in_maps = []
    for core in range(N_CORES):
        shard = primary_input[core * BC:(core + 1) * BC]          # [512, 8192]
        prim_t = np.ascontiguousarray(shard.T.astype(ml_dtypes.bfloat16))
        m = {"prim_t": prim_t}
        m.update(shared)
        in_maps.append(m)
    return in_maps


def _install_ntff_hook():
    """Provide antenv.axon_hooks (absent in this image) backed by ctypes
    calls into libaxon_pjrt.so, so run_bass_kernel_spmd(trace=True) can
    capture NTFF profiles. Mirrors trn_agent_boot.trn_boot."""
    import contextlib
    import ctypes
    import sys
    import types

    if "antenv.axon_hooks" in sys.modules:
        return
    so_path = "/opt/axon/libaxon_pjrt.so"
    lib = ctypes.CDLL(so_path)
    lib.axon_start_nrt_profile.argtypes = [ctypes.POINTER(ctypes.c_int64),
                                           ctypes.c_size_t]
    lib.axon_start_nrt_profile.restype = ctypes.c_int64
    lib.axon_stop_nrt_profile.argtypes = [ctypes.c_char_p]
    lib.axon_stop_nrt_profile.restype = ctypes.c_int64

    @contextlib.contextmanager
    def _hook(output_dir, device_ids):
        import jax
        jax.devices()
        if device_ids:
            ids = (ctypes.c_int64 * len(device_ids))(*device_ids)
            rc = lib.axon_start_nrt_profile(ids, len(device_ids))
        else:
            rc = lib.axon_start_nrt_profile(None, 0)
        if rc != 0:
            raise RuntimeError(f"axon_start_nrt_profile rc={rc}")
        try:
            yield
        finally:
            n = lib.axon_stop_nrt_profile(str(output_dir).encode())
            print(f"profile: {n} file(s) written to {output_dir}",
                  file=sys.stderr)

    mod = types.ModuleType("antenv.axon_hooks")
    mod.get_axon_ntff_profile_hook = lambda: _hook
    mod.set_axon_ntff_profile_hook = lambda h: None
    sys.modules["antenv.axon_hooks"] = mod
    import antenv
    antenv.axon_hooks = mod


def kernel(primary_input, W0, b0, W1, b1, W_out, b_out, _trace=False,
           _trace_cores=None):
    from concourse import bass_utils

    if _trace:
        _install_ntff_hook()

    nc, _ = _build_program()
    in_maps = _make_in_maps(primary_input, W0, b0, W1, b1, W_out, b_out)
    res = bass_utils.run_bass_kernel_spmd(
        nc, in_maps, core_ids=list(range(N_CORES)),
        trace=_trace, trace_cores=_trace_cores)
    out = np.empty((B_FULL, N_OUT), dtype=np.float32)
    for core in range(N_CORES):
        out[core * BC:(core + 1) * BC] = res.results[core]["out"].T
    if _trace:
        kernel._last_results = res
    return out



# revision 15
# speedup vs baseline: 1.8543x; 1.0006x over previous
"""Capsule-network kernel for 8x TRN2 NeuronCores (data-parallel over batch).

Reference computation (see problem):
  prim = primary_input.reshape(B, 8, 1024)
  prev = zeros(B, 4096)
  for col in 0..3:
    # layer0: inp = [prim_t, x_t, col] (1537) @ W0 -> relu -> flat -> roll(-128)
    # layer1: inp = [x_t, col] (513) @ W1 -> relu -> flat -> roll(+128)
  out = prev @ W_out + b_out

Kernel strategy (per core, batch shard Bc=512):
  - Everything on-chip is FEATURE-MAJOR: tiles are [128 features, Bc batch].
    ROLL=128 == partition count, so rolls are free tile re-indexings.
  - The scalar `col` concat input contributes col*W[last_row] to the
    pre-activation -> folded into per-col biases (computed on host).
  - P = prim @ W0[0:1024] is col-invariant -> computed once (phase 1),
    kept in SBUF as bf16, added during the layer0 drain each col.
  - col 0 layer0 has x=0 -> out = relu(P + b0): no matmuls at all.
  - Matmuls run as bf16 (same 1 col/cycle PE rate as fp32r, but
    the 2-byte LDWEIGHTS hides fully: measured cadence ~216ns vs
    ~227ns for fp32r).  Activations/weights bf16, psum fp32.
  - HW measurement: an fp32r matmul whose stationary weights differ from
    the previous matmul costs ~252ns; same-weights runs cost ~226.7ns.
    So layers are swept (fo, k) outer / row t inner: 8 consecutive
    matmuls share one weight tile (one sweep = 8 psum banks).
  - ~16 dummy matmuls at t=0 (on a memset tile) ramp the PE out of its
    low p-state during the initial DMA wait.
"""

import numpy as np

# ---- problem constants (hardcoded; kernel.py must be self-contained) ----
B_FULL = 4096
D_IN = 8192
T = 8            # NUM_TALL
NW = 4           # NUM_WIDE
F = 512          # feature size per capsule row
ROLL = 128
N_CORES = 8
BC = B_FULL // N_CORES   # per-core batch = 512
S = (F * T) // 128       # state feature tiles = 32
KP = (D_IN // T) // 128  # prim k-tiles per capsule row = 8
KX = F // 128            # x k-tiles = 4
FO = F // 128            # output feature tiles per row-layer = 4
N_OUT = 10
N_WARM = 12              # dummy p-state warmup matmuls

_CACHE = {}


def _build_program():
    """Build (and cache) the single-core Bass program. Same program runs
    SPMD on all 8 cores with different batch shards."""
    if "nc" in _CACHE:
        return _CACHE["nc"], _CACHE["names"]

    from contextlib import ExitStack

    import concourse.tile as tile
    from concourse import bacc, mybir

    f32 = mybir.dt.float32
    f32r = mybir.dt.float32r
    bf16 = mybir.dt.bfloat16
    AF = mybir.ActivationFunctionType
    ADD = mybir.AluOpType.add

    nc = bacc.Bacc("TRN2", target_bir_lowering=False, debug=False,
                   num_devices=N_CORES)

    # prim + w0p travel and multiply as bf16: phase-1 is the only
    # DMA-heavy span (16.8MB of prim in fp32 saturates the 360 GB/s DMA
    # system during the cold start); halving the bytes costs ~26ns/matmul
    # of bf16 LDWEIGHTS overhead on the 256 P-matmuls but removes all
    # DMA-starvation stalls.
    prim_d = nc.dram_tensor("prim_t", [D_IN, BC], bf16, kind="ExternalInput").ap()
    w0p_d = nc.dram_tensor("w0p", [KP * 128, F], bf16, kind="ExternalInput").ap()
    w0x_d = nc.dram_tensor("w0x", [F, F], bf16, kind="ExternalInput").ap()
    w1x_d = nc.dram_tensor("w1x", [F, F], bf16, kind="ExternalInput").ap()
    wout_d = nc.dram_tensor("wout_packed", [128, S * N_OUT], bf16,
                            kind="ExternalInput").ap()
    bias0_d = nc.dram_tensor("bias0", [128, NW * FO], f32, kind="ExternalInput").ap()
    bias1_d = nc.dram_tensor("bias1", [128, NW * FO], f32, kind="ExternalInput").ap()
    bout_d = nc.dram_tensor("bout", [N_OUT, 1], f32, kind="ExternalInput").ap()
    out_d = nc.dram_tensor("out", [N_OUT, BC], f32, kind="ExternalOutput").ap()

    with tile.TileContext(nc) as tc, ExitStack() as ctx:
        const = ctx.enter_context(tc.tile_pool(name="const", bufs=1))
        state = ctx.enter_context(tc.tile_pool(name="state", bufs=1))
        cpool = ctx.enter_context(tc.tile_pool(name="cpool", bufs=33))
        prim_pool = ctx.enter_context(tc.tile_pool(name="primp", bufs=12))
        ppool = ctx.enter_context(tc.tile_pool(name="psum", bufs=8, space="PSUM"))

        # ---- constants ----
        w0p_sb = [const.tile([128, F], bf16, name=f"w0p{k}", tag=f"w0p{k}")
                  for k in range(KP)]
        w0x_sb = [const.tile([128, F], bf16, name=f"w0x{k}", tag=f"w0x{k}")
                  for k in range(KX)]
        w1x_sb = [const.tile([128, F], bf16, name=f"w1x{k}", tag=f"w1x{k}")
                  for k in range(KX)]
        wout_sb = const.tile([128, S * N_OUT], bf16, name="wout_sb", tag="wout")
        bias0_sb = const.tile([128, NW * FO], f32, name="bias0_sb", tag="bias0")
        bias1_sb = const.tile([128, NW * FO], f32, name="bias1_sb", tag="bias1")
        bout_sb = const.tile([N_OUT, 1], f32, name="bout_sb", tag="bout")
        warm_sb = const.tile([128, 128], f32, name="warm_sb", tag="warm")

        # ---- persistent state ----
        A = [state.tile([128, BC], bf16, name=f"state_a{i}", tag=f"A{i}")
             for i in range(S)]
        P = [state.tile([128, BC], bf16, name=f"state_p{i}", tag=f"P{i}")
             for i in range(S)]

        # ---- p-state warmup: dummy matmuls on a memset tile ----
        # (fp32 runs at 4 cycles/row so a 128-wide moving dim gives
        # ~213-790ns per dummy across the ramp)
        nc.vector.memset(warm_sb[:], 0.0)
        for i in range(N_WARM):
            ps = ppool.tile([128, BC], f32, name=f"warm{i}", tag="mm")
            nc.tensor.matmul(ps[0:128, 0:128], warm_sb[:], warm_sb[:],
                             start=True, stop=True)

        def load_deferred_consts(gi):
            # late-needed constants ride the idle gpsimd (SWDGE) queue so
            # the sync/scalar queues keep streaming prim.  w1x must be
            # resident by ~15us (first col0-layer1 chunk), so it goes out
            # at gi=0 -- the gpsimd queue has nothing else and issues it
            # at t~1us.
            if gi == 0:
                nc.scalar.dma_start(bias0_sb[:], bias0_d[:, :])
                nc.gpsimd.dma_start(bias1_sb[:], bias1_d[:, :])
                nc.gpsimd.dma_start(bout_sb[:], bout_d[:, :])
                for k in range(KX):
                    nc.gpsimd.dma_start(w1x_sb[k][:], w1x_d[k * 128:(k + 1) * 128, :])
            elif gi == 1:
                for k in range(KX):
                    nc.gpsimd.dma_start(w0x_sb[k][:], w0x_d[k * 128:(k + 1) * 128, :])
            elif gi == 3:
                nc.gpsimd.dma_start(wout_sb[:], wout_d[:, :])

        C = [None] * S  # col-current layer0 outputs (cpool ring tiles)

        # ==== phase 1: P[j] = prim @ W0p (col-invariant), fused col0-L0 ====
        # Rows grouped [0], (1,2), (3,4), (5,6), [7]: within a group the
        # (k, fo) weight tile is reused across rows (run-of-2).
        # Drains: P copy (DVE, bf16) + col0-L0 C = relu(P+b0) (ACT).
        groups = [(0,), (1, 2), (3, 4), (5, 6), (7,)]

        def prim_dma(t, k):
            # one k-tile per DMA, full-width: the DMA system is
            # descriptor-line-rate limited, so 1KB lines (full bf16 rows)
            # move twice the bytes per line vs split halves.  Row 0 and
            # odd rows ride the scalar queue (it starts issuing ~5us
            # before sync, which carries the TileContext preamble).
            g = t * KP + k
            tile_ = prim_pool.tile([128, BC], bf16, name=f"prim_{g}",
                                   tag="prim")
            q = nc.scalar if (t == 0 or t % 2 == 1) else nc.sync
            q.dma_start(tile_[:], prim_d[g * 128:(g + 1) * 128, :])
            return tile_

        def layer1_chunk(c, rows):
            # layer1 for a subset of rows (weight run-of-len(rows)).
            # A[4t+fo] = relu(W1x.T C + b1c);  C k-tile = C[(4t+k+1)%S]
            for fo in range(FO):
                pss = {t: ppool.tile([128, BC], f32, name=f"ps1_{c}_{fo}_{t}",
                                     tag="mm") for t in rows}
                for k in range(KX):
                    w_ap = w1x_sb[k][:, fo * 128:(fo + 1) * 128]
                    for t in rows:
                        nc.tensor.matmul(
                            pss[t][:], w_ap, C[(t * FO + k + 1) % S][:],
                            start=(k == 0), stop=(k == KX - 1))
                b1ap = bias1_sb[:, c * FO + fo:c * FO + fo + 1]
                for t in rows:
                    j = t * FO + fo
                    if t % 2 == 0:
                        nc.scalar.activation(A[j][:], pss[t][:], AF.Relu,
                                             bias=b1ap)
                    else:
                        # relu(psum + bias) on DVE: (psum add bias) max 0
                        nc.vector.tensor_scalar(A[j][:], pss[t][:], b1ap, 0.0,
                                                ADD, mybir.AluOpType.max)

        for gi, grp in enumerate(groups):
            pss = {}
            for t in grp:
                for fo in range(FO):
                    pss[(t, fo)] = ppool.tile([128, BC], f32,
                                              name=f"ps_p1_{t}_{fo}", tag="mm")
            pt = {}
            if gi == 0:
                # interleave w0p with row 0's tiles on scalar so the
                # k-th matmul's pair (w0p[k], prim(0,k)) lands together
                for k in range(KP):
                    nc.scalar.dma_start(w0p_sb[k][:],
                                        w0p_d[k * 128:(k + 1) * 128, :])
                    pt[(0, k)] = prim_dma(0, k)
            else:
                for k in range(KP):
                    for t in grp:
                        pt[(t, k)] = prim_dma(t, k)
            for k in range(KP):
                for fo in range(FO):
                    for t in grp:
                        nc.tensor.matmul(
                            pss[(t, fo)][:],
                            w0p_sb[k][:, fo * 128:(fo + 1) * 128],
                            pt[(t, k)][:],
                            start=(k == 0), stop=(k == KP - 1))
            load_deferred_consts(gi)
            for t in grp:
                for fo in range(FO):
                    j = t * FO + fo
                    nc.vector.tensor_copy(P[j][:], pss[(t, fo)][:])
                    ct = cpool.tile([128, BC], bf16, name=f"c0_{j}", tag="C")
                    nc.scalar.activation(ct[:], pss[(t, fo)][:], AF.Relu,
                                         bias=bias0_sb[:, fo:fo + 1])
                    C[j] = ct
        # col-0 layer1 with full run-of-8 weight reuse (bf16 phase-1 DMA
        # leaves enough bandwidth slack that no absorber work is needed)
        layer1_chunk(0, tuple(range(T)))

        # ==== layer emitters: (fo, k) outer, t inner -> weight run-of-8 ====
        def layer0_col(c):
            # C[4t+fo] = relu(W0x.T x + P + b0c);  x k-tile = A[(4t+k-1)%S]
            # The t sweep starts at t=1: the k=0 input A[4t-1] is a fo3
            # tile of the previous col's layer1, and t=0 needs A[31] --
            # the very LAST drain of that col.  Rotating gives each A
            # one extra sweep-step of drain slack.
            rows = [(1 + i) % T for i in range(T)]
            for fo in range(FO):
                pss = {t: ppool.tile([128, BC], f32, name=f"ps0_{c}_{fo}_{t}",
                                     tag="mm") for t in rows}
                for k in range(KX):
                    w_ap = w0x_sb[k][:, fo * 128:(fo + 1) * 128]
                    for t in rows:
                        nc.tensor.matmul(
                            pss[t][:], w_ap, A[(t * FO + k - 1) % S][:],
                            start=(k == 0), stop=(k == KX - 1))
                b0ap = bias0_sb[:, c * FO + fo:c * FO + fo + 1]
                for t in rows:
                    j = t * FO + fo
                    ct = cpool.tile([128, BC], bf16, name=f"c{c}_{j}", tag="C")
                    # ct = (psum + bias0_c) + P  on DVE, then relu on ACT
                    nc.vector.scalar_tensor_tensor(
                        ct[:], pss[t][:], b0ap, P[j][:], ADD, ADD)
                    nc.scalar.activation(ct[:], ct[:], AF.Relu)
                    C[j] = ct

        def layer1_col(c):
            # A[4t+fo] = relu(W1x.T C + b1c);  C k-tile = C[(4t+k+1)%S]
            for fo in range(FO):
                pss = [ppool.tile([128, BC], f32, name=f"ps1_{c}_{fo}_{t}",
                                  tag="mm") for t in range(T)]
                for k in range(KX):
                    w_ap = w1x_sb[k][:, fo * 128:(fo + 1) * 128]
                    for t in range(T):
                        nc.tensor.matmul(
                            pss[t][:], w_ap, C[(t * FO + k + 1) % S][:],
                            start=(k == 0), stop=(k == KX - 1))
                b1ap = bias1_sb[:, c * FO + fo:c * FO + fo + 1]
                for t in range(T):
                    j = t * FO + fo
                    if t % 2 == 0:
                        nc.scalar.activation(A[j][:], pss[t][:], AF.Relu,
                                             bias=b1ap)
                    else:
                        # relu(psum + bias) on DVE: (psum add bias) max 0
                        nc.vector.tensor_scalar(A[j][:], pss[t][:], b1ap, 0.0,
                                                ADD, mybir.AluOpType.max)

        # ==== cols 1..3 (col-0 layer1 was interleaved into phase 1) ====
        for c in range(1, NW):
            layer0_col(c)
            layer1_col(c)

        # ---- final: out = prev @ W_out + b_out;  prev[k] = A[(k-1) % S] ----
        psf_full = ppool.tile([128, BC], f32, name="psf", tag="mm")
        psf = psf_full[0:N_OUT, :]
        # emit in col-3's A-write order (sweep fo, then t) so the
        # accumulation chain chases the layer1 drains
        n = 0
        for fo in range(FO):
            for t in range(T):
                k = (t * FO + fo + 1) % S
                nc.tensor.matmul(
                    psf[:],
                    wout_sb[:, k * N_OUT:(k + 1) * N_OUT],
                    A[(k - 1) % S][:],
                    start=(n == 0), stop=(n == S - 1))
                n += 1
        out_sb = cpool.tile([N_OUT, BC], f32, name="out_sb", tag="C")
        nc.scalar.activation(out_sb[:], psf[:], AF.Identity, bias=bout_sb[:])
        nc.sync.dma_start(out_d[:, :], out_sb[:])

    nc.compile()

    names = dict(prim="prim_t", w0p="w0p", w0x="w0x", w1x="w1x",
                 wout="wout_packed", bias0="bias0", bias1="bias1",
                 bout="bout", out="out")
    _CACHE["nc"] = nc
    _CACHE["names"] = names
    return nc, names


def _make_in_maps(primary_input, W0, b0, W1, b1, W_out, b_out):
    """Host-side sharding + layout prep (all cheap numpy except the
    feature-major transpose of the batch shards)."""
    primary_input = np.ascontiguousarray(primary_input, dtype=np.float32)
    W0 = np.asarray(W0, dtype=np.float32)
    b0 = np.asarray(b0, dtype=np.float32)
    W1 = np.asarray(W1, dtype=np.float32)
    b1 = np.asarray(b1, dtype=np.float32)
    W_out = np.asarray(W_out, dtype=np.float32)
    b_out = np.asarray(b_out, dtype=np.float32)

    import ml_dtypes
    ps = D_IN // T  # 1024
    w0p = np.ascontiguousarray(W0[:ps].astype(ml_dtypes.bfloat16))  # [1024, 512]
    w0x = np.ascontiguousarray(W0[ps:ps + F].astype(ml_dtypes.bfloat16))
    w0_last = W0[ps + F]                             # [512]
    w1x = np.ascontiguousarray(W1[:F].astype(ml_dtypes.bfloat16))
    w1_last = W1[F]                                  # [512]

    bias0 = np.concatenate(
        [(b0 + c * w0_last).reshape(FO, 128).T for c in range(NW)], axis=1)
    bias1 = np.concatenate(
        [(b1 + c * w1_last).reshape(FO, 128).T for c in range(NW)], axis=1)
    bias0 = np.ascontiguousarray(bias0, dtype=np.float32)   # [128, 16]
    bias1 = np.ascontiguousarray(bias1, dtype=np.float32)   # [128, 16]

    # wout_packed[p, k*10+o] = W_out[128k+p, o]
    wout_packed = np.ascontiguousarray(
        W_out.reshape(S, 128, N_OUT).transpose(1, 0, 2).reshape(128, S * N_OUT)
        .astype(ml_dtypes.bfloat16))
    bout = np.ascontiguousarray(b_out.reshape(N_OUT, 1))

    shared = dict(w0p=w0p, w0x=w0x, w1x=w1x, wout_packed=wout_packed,
                  bias0=bias0, bias1=bias1, bout=bout)
    in_maps = []
    for core in range(N_CORES):
        shard = primary_input[core * BC:(core + 1) * BC]          # [512, 8192]
        prim_t = np.ascontiguousarray(shard.T.astype(ml_dtypes.bfloat16))
        m = {"prim_t": prim_t}
        m.update(shared)
        in_maps.append(m)
    return in_maps


def _install_ntff_hook():
    """Provide antenv.axon_hooks (absent in this image) backed by ctypes
    calls into libaxon_pjrt.so, so run_bass_kernel_spmd(trace=True) can
    capture NTFF profiles. Mirrors trn_agent_boot.trn_boot."""
    import contextlib
    import ctypes
    import sys
    import types

    if "antenv.axon_hooks" in sys.modules:
        return
    so_path = "/opt/axon/libaxon_pjrt.so"
    lib = ctypes.CDLL(so_path)
    lib.axon_start_nrt_profile.argtypes = [ctypes.POINTER(ctypes.c_int64),
                                           ctypes.c_size_t]
    lib.axon_start_nrt_profile.restype = ctypes.c_int64
    lib.axon_stop_nrt_profile.argtypes = [ctypes.c_char_p]
    lib.axon_stop_nrt_profile.restype = ctypes.c_int64

    @contextlib.contextmanager
    def _hook(output_dir, device_ids):
        import jax
        jax.devices()
        if device_ids:
            ids = (ctypes.c_int64 * len(device_ids))(*device_ids)
            rc = lib.axon_start_nrt_profile(ids, len(device_ids))
        else:
            rc = lib.axon_start_nrt_profile(None, 0)
        if rc != 0:
            raise RuntimeError(f"axon_start_nrt_profile rc={rc}")
        try:
            yield
        finally:
            n = lib.axon_stop_nrt_profile(str(output_dir).encode())
            print(f"profile: {n} file(s) written to {output_dir}",
                  file=sys.stderr)

    mod = types.ModuleType("antenv.axon_hooks")
    mod.get_axon_ntff_profile_hook = lambda: _hook
    mod.set_axon_ntff_profile_hook = lambda h: None
    sys.modules["antenv.axon_hooks"] = mod
    import antenv
    antenv.axon_hooks = mod


def kernel(primary_input, W0, b0, W1, b1, W_out, b_out, _trace=False,
           _trace_cores=None):
    from concourse import bass_utils

    if _trace:
        _install_ntff_hook()

    nc, _ = _build_program()
    in_maps = _make_in_maps(primary_input, W0, b0, W1, b1, W_out, b_out)
    res = bass_utils.run_bass_kernel_spmd(
        nc, in_maps, core_ids=list(range(N_CORES)),
        trace=_trace, trace_cores=_trace_cores)
    out = np.empty((B_FULL, N_OUT), dtype=np.float32)
    for core in range(N_CORES):
        out[core * BC:(core + 1) * BC] = res.results[core]["out"].T
    if _trace:
        kernel._last_results = res
    return out



# revision 16
# speedup vs baseline: 1.8563x; 1.0011x over previous
"""Capsule-network kernel for 8x TRN2 NeuronCores (data-parallel over batch).

Reference computation (see problem):
  prim = primary_input.reshape(B, 8, 1024)
  prev = zeros(B, 4096)
  for col in 0..3:
    # layer0: inp = [prim_t, x_t, col] (1537) @ W0 -> relu -> flat -> roll(-128)
    # layer1: inp = [x_t, col] (513) @ W1 -> relu -> flat -> roll(+128)
  out = prev @ W_out + b_out

Kernel strategy (per core, batch shard Bc=512):
  - Everything on-chip is FEATURE-MAJOR: tiles are [128 features, Bc batch].
    ROLL=128 == partition count, so rolls are free tile re-indexings.
  - The scalar `col` concat input contributes col*W[last_row] to the
    pre-activation -> folded into per-col biases (computed on host).
  - P = prim @ W0[0:1024] is col-invariant -> computed once (phase 1),
    kept in SBUF as bf16, added during the layer0 drain each col.
  - col 0 layer0 has x=0 -> out = relu(P + b0): no matmuls at all.
  - Matmuls run as bf16 (same 1 col/cycle PE rate as fp32r, but
    the 2-byte LDWEIGHTS hides fully: measured cadence ~216ns vs
    ~227ns for fp32r).  Activations/weights bf16, psum fp32.
  - HW measurement: an fp32r matmul whose stationary weights differ from
    the previous matmul costs ~252ns; same-weights runs cost ~226.7ns.
    So layers are swept (fo, k) outer / row t inner: 8 consecutive
    matmuls share one weight tile (one sweep = 8 psum banks).
  - ~16 dummy matmuls at t=0 (on a memset tile) ramp the PE out of its
    low p-state during the initial DMA wait.
"""

import numpy as np

# ---- problem constants (hardcoded; kernel.py must be self-contained) ----
B_FULL = 4096
D_IN = 8192
T = 8            # NUM_TALL
NW = 4           # NUM_WIDE
F = 512          # feature size per capsule row
ROLL = 128
N_CORES = 8
BC = B_FULL // N_CORES   # per-core batch = 512
S = (F * T) // 128       # state feature tiles = 32
KP = (D_IN // T) // 128  # prim k-tiles per capsule row = 8
KX = F // 128            # x k-tiles = 4
FO = F // 128            # output feature tiles per row-layer = 4
N_OUT = 10
N_WARM = 12              # dummy p-state warmup matmuls

_CACHE = {}


def _build_program():
    """Build (and cache) the single-core Bass program. Same program runs
    SPMD on all 8 cores with different batch shards."""
    if "nc" in _CACHE:
        return _CACHE["nc"], _CACHE["names"]

    from contextlib import ExitStack

    import concourse.tile as tile
    from concourse import bacc, mybir

    f32 = mybir.dt.float32
    f32r = mybir.dt.float32r
    bf16 = mybir.dt.bfloat16
    AF = mybir.ActivationFunctionType
    ADD = mybir.AluOpType.add

    nc = bacc.Bacc("TRN2", target_bir_lowering=False, debug=False,
                   num_devices=N_CORES)

    # prim + w0p travel and multiply as bf16: phase-1 is the only
    # DMA-heavy span (16.8MB of prim in fp32 saturates the 360 GB/s DMA
    # system during the cold start); halving the bytes costs ~26ns/matmul
    # of bf16 LDWEIGHTS overhead on the 256 P-matmuls but removes all
    # DMA-starvation stalls.
    prim_d = nc.dram_tensor("prim_t", [D_IN, BC], bf16, kind="ExternalInput").ap()
    w0p_d = nc.dram_tensor("w0p", [KP * 128, F], bf16, kind="ExternalInput").ap()
    w0x_d = nc.dram_tensor("w0x", [F, F], bf16, kind="ExternalInput").ap()
    w1x_d = nc.dram_tensor("w1x", [F, F], bf16, kind="ExternalInput").ap()
    wout_d = nc.dram_tensor("wout_packed", [128, S * N_OUT], bf16,
                            kind="ExternalInput").ap()
    bias0_d = nc.dram_tensor("bias0", [128, NW * FO], f32, kind="ExternalInput").ap()
    bias1_d = nc.dram_tensor("bias1", [128, NW * FO], f32, kind="ExternalInput").ap()
    bout_d = nc.dram_tensor("bout", [N_OUT, 1], f32, kind="ExternalInput").ap()
    out_d = nc.dram_tensor("out", [N_OUT, BC], f32, kind="ExternalOutput").ap()

    with tile.TileContext(nc) as tc, ExitStack() as ctx:
        const = ctx.enter_context(tc.tile_pool(name="const", bufs=1))
        state = ctx.enter_context(tc.tile_pool(name="state", bufs=1))
        cpool = ctx.enter_context(tc.tile_pool(name="cpool", bufs=33))
        prim_pool = ctx.enter_context(tc.tile_pool(name="primp", bufs=12))
        ppool = ctx.enter_context(tc.tile_pool(name="psum", bufs=8, space="PSUM"))

        # ---- constants ----
        w0p_sb = [const.tile([128, F], bf16, name=f"w0p{k}", tag=f"w0p{k}")
                  for k in range(KP)]
        w0x_sb = [const.tile([128, F], bf16, name=f"w0x{k}", tag=f"w0x{k}")
                  for k in range(KX)]
        w1x_sb = [const.tile([128, F], bf16, name=f"w1x{k}", tag=f"w1x{k}")
                  for k in range(KX)]
        wout_sb = const.tile([128, S * N_OUT], bf16, name="wout_sb", tag="wout")
        bias0_sb = const.tile([128, NW * FO], f32, name="bias0_sb", tag="bias0")
        bias1_sb = const.tile([128, NW * FO], f32, name="bias1_sb", tag="bias1")
        bout_sb = const.tile([N_OUT, 1], f32, name="bout_sb", tag="bout")
        warm_sb = const.tile([128, 128], f32, name="warm_sb", tag="warm")

        # ---- persistent state ----
        A = [state.tile([128, BC], bf16, name=f"state_a{i}", tag=f"A{i}")
             for i in range(S)]
        P = [state.tile([128, BC], bf16, name=f"state_p{i}", tag=f"P{i}")
             for i in range(S)]

        # ---- p-state warmup: dummy matmuls on a memset tile ----
        # (fp32 runs at 4 cycles/row so a 128-wide moving dim gives
        # ~213-790ns per dummy across the ramp)
        nc.vector.memset(warm_sb[:], 0.0)
        for i in range(N_WARM):
            ps = ppool.tile([128, BC], f32, name=f"warm{i}", tag="mm")
            nc.tensor.matmul(ps[0:128, 0:128], warm_sb[:], warm_sb[:],
                             start=True, stop=True)

        def load_deferred_consts(gi):
            # late-needed constants ride the idle gpsimd (SWDGE) queue so
            # the sync/scalar queues keep streaming prim.  w1x must be
            # resident by ~15us (first col0-layer1 chunk), so it goes out
            # at gi=0 -- the gpsimd queue has nothing else and issues it
            # at t~1us.
            if gi == 0:
                nc.scalar.dma_start(bias0_sb[:], bias0_d[:, :])
                nc.gpsimd.dma_start(bias1_sb[:], bias1_d[:, :])
                nc.gpsimd.dma_start(bout_sb[:], bout_d[:, :])
                for k in range(KX):
                    nc.gpsimd.dma_start(w1x_sb[k][:], w1x_d[k * 128:(k + 1) * 128, :])
            elif gi == 1:
                for k in range(KX):
                    nc.gpsimd.dma_start(w0x_sb[k][:], w0x_d[k * 128:(k + 1) * 128, :])
            elif gi == 3:
                nc.gpsimd.dma_start(wout_sb[:], wout_d[:, :])

        C = [None] * S  # col-current layer0 outputs (cpool ring tiles)

        # ==== phase 1: P[j] = prim @ W0p (col-invariant), fused col0-L0 ====
        # Rows grouped [0], (1,2), (3,4), (5,6), [7]: within a group the
        # (k, fo) weight tile is reused across rows (run-of-2).
        # Drains: P copy (DVE, bf16) + col0-L0 C = relu(P+b0) (ACT).
        groups = [(0,), (1, 2), (3, 4), (5, 6), (7,)]

        def prim_dma(t, k):
            # one k-tile per DMA, full-width: the DMA system is
            # descriptor-line-rate limited, so 1KB lines (full bf16 rows)
            # move twice the bytes per line vs split halves.  Row 0 and
            # odd rows ride the scalar queue (it starts issuing ~5us
            # before sync, which carries the TileContext preamble).
            g = t * KP + k
            tile_ = prim_pool.tile([128, BC], bf16, name=f"prim_{g}",
                                   tag="prim")
            # scalar only carries rows 0-1: its engine (ACT) also runs the
            # phase-1 relu drains, and DMA issue costs ~630ns each.  Rows
            # 3/5/7 ride the otherwise-idle gpsimd queue (after the small
            # deferred consts), rows 2/4/6 the sync queue.
            if t <= 1:
                q = nc.scalar
            elif t % 2 == 0:
                q = nc.sync
            else:
                q = nc.gpsimd
            q.dma_start(tile_[:], prim_d[g * 128:(g + 1) * 128, :])
            return tile_

        def layer1_chunk(c, rows):
            # layer1 for a subset of rows (weight run-of-len(rows)).
            # A[4t+fo] = relu(W1x.T C + b1c);  C k-tile = C[(4t+k+1)%S]
            for fo in range(FO):
                pss = {t: ppool.tile([128, BC], f32, name=f"ps1_{c}_{fo}_{t}",
                                     tag="mm") for t in rows}
                for k in range(KX):
                    w_ap = w1x_sb[k][:, fo * 128:(fo + 1) * 128]
                    for t in rows:
                        nc.tensor.matmul(
                            pss[t][:], w_ap, C[(t * FO + k + 1) % S][:],
                            start=(k == 0), stop=(k == KX - 1))
                b1ap = bias1_sb[:, c * FO + fo:c * FO + fo + 1]
                for t in rows:
                    j = t * FO + fo
                    if t % 2 == 0:
                        nc.scalar.activation(A[j][:], pss[t][:], AF.Relu,
                                             bias=b1ap)
                    else:
                        # relu(psum + bias) on DVE: (psum add bias) max 0
                        nc.vector.tensor_scalar(A[j][:], pss[t][:], b1ap, 0.0,
                                                ADD, mybir.AluOpType.max)

        for gi, grp in enumerate(groups):
            pss = {}
            for t in grp:
                for fo in range(FO):
                    pss[(t, fo)] = ppool.tile([128, BC], f32,
                                              name=f"ps_p1_{t}_{fo}", tag="mm")
            pt = {}
            if gi == 0:
                # interleave w0p with row 0's tiles on scalar so the
                # k-th matmul's pair (w0p[k], prim(0,k)) lands together
                for k in range(KP):
                    nc.scalar.dma_start(w0p_sb[k][:],
                                        w0p_d[k * 128:(k + 1) * 128, :])
                    pt[(0, k)] = prim_dma(0, k)
            else:
                for k in range(KP):
                    for t in grp:
                        pt[(t, k)] = prim_dma(t, k)
            for k in range(KP):
                for fo in range(FO):
                    for t in grp:
                        nc.tensor.matmul(
                            pss[(t, fo)][:],
                            w0p_sb[k][:, fo * 128:(fo + 1) * 128],
                            pt[(t, k)][:],
                            start=(k == 0), stop=(k == KP - 1))
            load_deferred_consts(gi)
            for t in grp:
                for fo in range(FO):
                    j = t * FO + fo
                    nc.vector.tensor_copy(P[j][:], pss[(t, fo)][:])
                    ct = cpool.tile([128, BC], bf16, name=f"c0_{j}", tag="C")
                    nc.scalar.activation(ct[:], pss[(t, fo)][:], AF.Relu,
                                         bias=bias0_sb[:, fo:fo + 1])
                    C[j] = ct
        # col-0 layer1 with full run-of-8 weight reuse (bf16 phase-1 DMA
        # leaves enough bandwidth slack that no absorber work is needed)
        layer1_chunk(0, tuple(range(T)))

        # ==== layer emitters: (fo, k) outer, t inner -> weight run-of-8 ====
        def layer0_col(c):
            # C[4t+fo] = relu(W0x.T x + P + b0c);  x k-tile = A[(4t+k-1)%S]
            # The t sweep starts at t=1: the k=0 input A[4t-1] is a fo3
            # tile of the previous col's layer1, and t=0 needs A[31] --
            # the very LAST drain of that col.  Rotating gives each A
            # one extra sweep-step of drain slack.
            rows = [(1 + i) % T for i in range(T)]
            for fo in range(FO):
                pss = {t: ppool.tile([128, BC], f32, name=f"ps0_{c}_{fo}_{t}",
                                     tag="mm") for t in rows}
                for k in range(KX):
                    w_ap = w0x_sb[k][:, fo * 128:(fo + 1) * 128]
                    for t in rows:
                        nc.tensor.matmul(
                            pss[t][:], w_ap, A[(t * FO + k - 1) % S][:],
                            start=(k == 0), stop=(k == KX - 1))
                b0ap = bias0_sb[:, c * FO + fo:c * FO + fo + 1]
                for t in rows:
                    j = t * FO + fo
                    ct = cpool.tile([128, BC], bf16, name=f"c{c}_{j}", tag="C")
                    # ct = (psum + bias0_c) + P  on DVE, then relu on ACT
                    nc.vector.scalar_tensor_tensor(
                        ct[:], pss[t][:], b0ap, P[j][:], ADD, ADD)
                    nc.scalar.activation(ct[:], ct[:], AF.Relu)
                    C[j] = ct

        def layer1_col(c):
            # A[4t+fo] = relu(W1x.T C + b1c);  C k-tile = C[(4t+k+1)%S]
            for fo in range(FO):
                pss = [ppool.tile([128, BC], f32, name=f"ps1_{c}_{fo}_{t}",
                                  tag="mm") for t in range(T)]
                for k in range(KX):
                    w_ap = w1x_sb[k][:, fo * 128:(fo + 1) * 128]
                    for t in range(T):
                        nc.tensor.matmul(
                            pss[t][:], w_ap, C[(t * FO + k + 1) % S][:],
                            start=(k == 0), stop=(k == KX - 1))
                b1ap = bias1_sb[:, c * FO + fo:c * FO + fo + 1]
                for t in range(T):
                    j = t * FO + fo
                    if t % 2 == 0:
                        nc.scalar.activation(A[j][:], pss[t][:], AF.Relu,
                                             bias=b1ap)
                    else:
                        # relu(psum + bias) on DVE: (psum add bias) max 0
                        nc.vector.tensor_scalar(A[j][:], pss[t][:], b1ap, 0.0,
                                                ADD, mybir.AluOpType.max)

        # ==== cols 1..3 (col-0 layer1 was interleaved into phase 1) ====
        for c in range(1, NW):
            layer0_col(c)
            layer1_col(c)

        # ---- final: out = prev @ W_out + b_out;  prev[k] = A[(k-1) % S] ----
        psf_full = ppool.tile([128, BC], f32, name="psf", tag="mm")
        psf = psf_full[0:N_OUT, :]
        # emit in col-3's A-write order (sweep fo, then t) so the
        # accumulation chain chases the layer1 drains
        n = 0
        for fo in range(FO):
            for t in range(T):
                k = (t * FO + fo + 1) % S
                nc.tensor.matmul(
                    psf[:],
                    wout_sb[:, k * N_OUT:(k + 1) * N_OUT],
                    A[(k - 1) % S][:],
                    start=(n == 0), stop=(n == S - 1))
                n += 1
        out_sb = cpool.tile([N_OUT, BC], f32, name="out_sb", tag="C")
        nc.scalar.activation(out_sb[:], psf[:], AF.Identity, bias=bout_sb[:])
        nc.sync.dma_start(out_d[:, :], out_sb[:])

    nc.compile()

    names = dict(prim="prim_t", w0p="w0p", w0x="w0x", w1x="w1x",
                 wout="wout_packed", bias0="bias0", bias1="bias1",
                 bout="bout", out="out")
    _CACHE["nc"] = nc
    _CACHE["names"] = names
    return nc, names


def _make_in_maps(primary_input, W0, b0, W1, b1, W_out, b_out):
    """Host-side sharding + layout prep (all cheap numpy except the
    feature-major transpose of the batch shards)."""
    primary_input = np.ascontiguousarray(primary_input, dtype=np.float32)
    W0 = np.asarray(W0, dtype=np.float32)
    b0 = np.asarray(b0, dtype=np.float32)
    W1 = np.asarray(W1, dtype=np.float32)
    b1 = np.asarray(b1, dtype=np.float32)
    W_out = np.asarray(W_out, dtype=np.float32)
    b_out = np.asarray(b_out, dtype=np.float32)

    import ml_dtypes
    ps = D_IN // T  # 1024
    w0p = np.ascontiguousarray(W0[:ps].astype(ml_dtypes.bfloat16))  # [1024, 512]
    w0x = np.ascontiguousarray(W0[ps:ps + F].astype(ml_dtypes.bfloat16))
    w0_last = W0[ps + F]                             # [512]
    w1x = np.ascontiguousarray(W1[:F].astype(ml_dtypes.bfloat16))
    w1_last = W1[F]                                  # [512]

    bias0 = np.concatenate(
        [(b0 + c * w0_last).reshape(FO, 128).T for c in range(NW)], axis=1)
    bias1 = np.concatenate(
        [(b1 + c * w1_last).reshape(FO, 128).T for c in range(NW)], axis=1)
    bias0 = np.ascontiguousarray(bias0, dtype=np.float32)   # [128, 16]
    bias1 = np.ascontiguousarray(bias1, dtype=np.float32)   # [128, 16]

    # wout_packed[p, k*10+o] = W_out[128k+p, o]
    wout_packed = np.ascontiguousarray(
        W_out.reshape(S, 128, N_OUT).transpose(1, 0, 2).reshape(128, S * N_OUT)
        .astype(ml_dtypes.bfloat16))
    bout = np.ascontiguousarray(b_out.reshape(N_OUT, 1))

    shared = dict(w0p=w0p, w0x=w0x, w1x=w1x, wout_packed=wout_packed,
                  bias0=bias0, bias1=bias1, bout=bout)
    in_maps = []
    for core in range(N_CORES):
        shard = primary_input[core * BC:(core + 1) * BC]          # [512, 8192]
        prim_t = np.ascontiguousarray(shard.T.astype(ml_dtypes.bfloat16))
        m = {"prim_t": prim_t}
        m.update(shared)
        in_maps.append(m)
    return in_maps


def _install_ntff_hook():
    """Provide antenv.axon_hooks (absent in this image) backed by ctypes
    calls into libaxon_pjrt.so, so run_bass_kernel_spmd(trace=True) can
    capture NTFF profiles. Mirrors trn_agent_boot.trn_boot."""
    import contextlib
    import ctypes
    import sys
    import types

    if "antenv.axon_hooks" in sys.modules:
        return
    so_path = "/opt/axon/libaxon_pjrt.so"
    lib = ctypes.CDLL(so_path)
    lib.axon_start_nrt_profile.argtypes = [ctypes.POINTER(ctypes.c_int64),
                                           ctypes.c_size_t]
    lib.axon_start_nrt_profile.restype = ctypes.c_int64
    lib.axon_stop_nrt_profile.argtypes = [ctypes.c_char_p]
    lib.axon_stop_nrt_profile.restype = ctypes.c_int64

    @contextlib.contextmanager
    def _hook(output_dir, device_ids):
        import jax
        jax.devices()
        if device_ids:
            ids = (ctypes.c_int64 * len(device_ids))(*device_ids)
            rc = lib.axon_start_nrt_profile(ids, len(device_ids))
        else:
            rc = lib.axon_start_nrt_profile(None, 0)
        if rc != 0:
            raise RuntimeError(f"axon_start_nrt_profile rc={rc}")
        try:
            yield
        finally:
            n = lib.axon_stop_nrt_profile(str(output_dir).encode())
            print(f"profile: {n} file(s) written to {output_dir}",
                  file=sys.stderr)

    mod = types.ModuleType("antenv.axon_hooks")
    mod.get_axon_ntff_profile_hook = lambda: _hook
    mod.set_axon_ntff_profile_hook = lambda h: None
    sys.modules["antenv.axon_hooks"] = mod
    import antenv
    antenv.axon_hooks = mod


def kernel(primary_input, W0, b0, W1, b1, W_out, b_out, _trace=False,
           _trace_cores=None):
    from concourse import bass_utils

    if _trace:
        _install_ntff_hook()

    nc, _ = _build_program()
    in_maps = _make_in_maps(primary_input, W0, b0, W1, b1, W_out, b_out)
    res = bass_utils.run_bass_kernel_spmd(
        nc, in_maps, core_ids=list(range(N_CORES)),
        trace=_trace, trace_cores=_trace_cores)
    out = np.empty((B_FULL, N_OUT), dtype=np.float32)
    for core in range(N_CORES):
        out[core * BC:(core + 1) * BC] = res.results[core]["out"].T
    if _trace:
        kernel._last_results = res
    return out



# revision 17
# speedup vs baseline: 1.8575x; 1.0006x over previous
"""Capsule-network kernel for 8x TRN2 NeuronCores (data-parallel over batch).

Reference computation (see problem):
  prim = primary_input.reshape(B, 8, 1024)
  prev = zeros(B, 4096)
  for col in 0..3:
    # layer0: inp = [prim_t, x_t, col] (1537) @ W0 -> relu -> flat -> roll(-128)
    # layer1: inp = [x_t, col] (513) @ W1 -> relu -> flat -> roll(+128)
  out = prev @ W_out + b_out

Kernel strategy (per core, batch shard Bc=512):
  - Everything on-chip is FEATURE-MAJOR: tiles are [128 features, Bc batch].
    ROLL=128 == partition count, so rolls are free tile re-indexings.
  - The scalar `col` concat input contributes col*W[last_row] to the
    pre-activation -> folded into per-col biases (computed on host).
  - P = prim @ W0[0:1024] is col-invariant -> computed once (phase 1),
    kept in SBUF as bf16, added during the layer0 drain each col.
  - col 0 layer0 has x=0 -> out = relu(P + b0): no matmuls at all.
  - Matmuls run as bf16 (same 1 col/cycle PE rate as fp32r, but
    the 2-byte LDWEIGHTS hides fully: measured cadence ~216ns vs
    ~227ns for fp32r).  Activations/weights bf16, psum fp32.
  - HW measurement: an fp32r matmul whose stationary weights differ from
    the previous matmul costs ~252ns; same-weights runs cost ~226.7ns.
    So layers are swept (fo, k) outer / row t inner: 8 consecutive
    matmuls share one weight tile (one sweep = 8 psum banks).
  - ~16 dummy matmuls at t=0 (on a memset tile) ramp the PE out of its
    low p-state during the initial DMA wait.
"""

import numpy as np

# ---- problem constants (hardcoded; kernel.py must be self-contained) ----
B_FULL = 4096
D_IN = 8192
T = 8            # NUM_TALL
NW = 4           # NUM_WIDE
F = 512          # feature size per capsule row
ROLL = 128
N_CORES = 8
BC = B_FULL // N_CORES   # per-core batch = 512
S = (F * T) // 128       # state feature tiles = 32
KP = (D_IN // T) // 128  # prim k-tiles per capsule row = 8
KX = F // 128            # x k-tiles = 4
FO = F // 128            # output feature tiles per row-layer = 4
N_OUT = 10
N_WARM = 6               # dummy p-state warmup matmuls

_CACHE = {}


def _build_program():
    """Build (and cache) the single-core Bass program. Same program runs
    SPMD on all 8 cores with different batch shards."""
    if "nc" in _CACHE:
        return _CACHE["nc"], _CACHE["names"]

    from contextlib import ExitStack

    import concourse.tile as tile
    from concourse import bacc, mybir

    f32 = mybir.dt.float32
    f32r = mybir.dt.float32r
    bf16 = mybir.dt.bfloat16
    AF = mybir.ActivationFunctionType
    ADD = mybir.AluOpType.add

    nc = bacc.Bacc("TRN2", target_bir_lowering=False, debug=False,
                   num_devices=N_CORES)

    # prim + w0p travel and multiply as bf16: phase-1 is the only
    # DMA-heavy span (16.8MB of prim in fp32 saturates the 360 GB/s DMA
    # system during the cold start); halving the bytes costs ~26ns/matmul
    # of bf16 LDWEIGHTS overhead on the 256 P-matmuls but removes all
    # DMA-starvation stalls.
    prim_d = nc.dram_tensor("prim_t", [D_IN, BC], bf16, kind="ExternalInput").ap()
    w0p_d = nc.dram_tensor("w0p", [KP * 128, F], bf16, kind="ExternalInput").ap()
    w0x_d = nc.dram_tensor("w0x", [F, F], bf16, kind="ExternalInput").ap()
    w1x_d = nc.dram_tensor("w1x", [F, F], bf16, kind="ExternalInput").ap()
    wout_d = nc.dram_tensor("wout_packed", [128, S * N_OUT], bf16,
                            kind="ExternalInput").ap()
    bias0_d = nc.dram_tensor("bias0", [128, NW * FO], f32, kind="ExternalInput").ap()
    bias1_d = nc.dram_tensor("bias1", [128, NW * FO], f32, kind="ExternalInput").ap()
    bout_d = nc.dram_tensor("bout", [N_OUT, 1], f32, kind="ExternalInput").ap()
    out_d = nc.dram_tensor("out", [N_OUT, BC], f32, kind="ExternalOutput").ap()

    with tile.TileContext(nc) as tc, ExitStack() as ctx:
        const = ctx.enter_context(tc.tile_pool(name="const", bufs=1))
        state = ctx.enter_context(tc.tile_pool(name="state", bufs=1))
        cpool = ctx.enter_context(tc.tile_pool(name="cpool", bufs=33))
        prim_pool = ctx.enter_context(tc.tile_pool(name="primp", bufs=12))
        ppool = ctx.enter_context(tc.tile_pool(name="psum", bufs=8, space="PSUM"))

        # ---- constants ----
        w0p_sb = [const.tile([128, F], bf16, name=f"w0p{k}", tag=f"w0p{k}")
                  for k in range(KP)]
        w0x_sb = [const.tile([128, F], bf16, name=f"w0x{k}", tag=f"w0x{k}")
                  for k in range(KX)]
        w1x_sb = [const.tile([128, F], bf16, name=f"w1x{k}", tag=f"w1x{k}")
                  for k in range(KX)]
        wout_sb = const.tile([128, S * N_OUT], bf16, name="wout_sb", tag="wout")
        bias0_sb = const.tile([128, NW * FO], f32, name="bias0_sb", tag="bias0")
        bias1_sb = const.tile([128, NW * FO], f32, name="bias1_sb", tag="bias1")
        bout_sb = const.tile([N_OUT, 1], f32, name="bout_sb", tag="bout")
        warm_sb = const.tile([128, 128], f32, name="warm_sb", tag="warm")

        # ---- persistent state ----
        A = [state.tile([128, BC], bf16, name=f"state_a{i}", tag=f"A{i}")
             for i in range(S)]
        P = [state.tile([128, BC], bf16, name=f"state_p{i}", tag=f"P{i}")
             for i in range(S)]

        # ---- p-state warmup: dummy matmuls on a memset tile ----
        # (fp32 runs at 4 cycles/row so a 128-wide moving dim gives
        # ~213-790ns per dummy across the ramp)
        nc.vector.memset(warm_sb[:], 0.0)
        for i in range(N_WARM):
            ps = ppool.tile([128, BC], f32, name=f"warm{i}", tag="mm")
            nc.tensor.matmul(ps[0:128, 0:128], warm_sb[:], warm_sb[:],
                             start=True, stop=True)

        def load_deferred_consts(gi):
            # late-needed constants ride the idle gpsimd (SWDGE) queue so
            # the sync/scalar queues keep streaming prim.  w1x must be
            # resident by ~15us (first col0-layer1 chunk), so it goes out
            # at gi=0 -- the gpsimd queue has nothing else and issues it
            # at t~1us.
            if gi == 0:
                nc.sync.dma_start(bias0_sb[:], bias0_d[:, :])
                nc.gpsimd.dma_start(bias1_sb[:], bias1_d[:, :])
                nc.gpsimd.dma_start(bout_sb[:], bout_d[:, :])
                for k in range(KX):
                    nc.gpsimd.dma_start(w1x_sb[k][:], w1x_d[k * 128:(k + 1) * 128, :])
            elif gi == 1:
                for k in range(KX):
                    nc.gpsimd.dma_start(w0x_sb[k][:], w0x_d[k * 128:(k + 1) * 128, :])
            elif gi == 5:
                nc.gpsimd.dma_start(wout_sb[:], wout_d[:, :])

        C = [None] * S  # col-current layer0 outputs (cpool ring tiles)

        # ==== phase 1: P[j] = prim @ W0p (col-invariant), fused col0-L0 ====
        # Rows grouped [0], (1,2), (3,4), (5,6), [7]: within a group the
        # (k, fo) weight tile is reused across rows (run-of-2).
        # Drains: P copy (DVE, bf16) + col0-L0 C = relu(P+b0) (ACT).
        groups = [(t,) for t in range(T)]

        def prim_dma(t, k):
            # one k-tile per DMA, full-width: the DMA system is
            # descriptor-line-rate limited, so 1KB lines (full bf16 rows)
            # move twice the bytes per line vs split halves.  Row 0 and
            # odd rows ride the scalar queue (it starts issuing ~5us
            # before sync, which carries the TileContext preamble).
            g = t * KP + k
            tile_ = prim_pool.tile([128, BC], bf16, name=f"prim_{g}",
                                   tag="prim")
            # scalar only carries rows 0-1: its engine (ACT) also runs the
            # phase-1 relu drains, and DMA issue costs ~630ns each.  Rows
            # 3/5/7 ride the otherwise-idle gpsimd queue (after the small
            # deferred consts), rows 2/4/6 the sync queue.
            if t <= 1:
                q = nc.scalar
            elif t % 2 == 0:
                q = nc.sync
            else:
                q = nc.gpsimd
            q.dma_start(tile_[:], prim_d[g * 128:(g + 1) * 128, :])
            return tile_

        def layer1_chunk(c, rows):
            # layer1 for a subset of rows (weight run-of-len(rows)).
            # A[4t+fo] = relu(W1x.T C + b1c);  C k-tile = C[(4t+k+1)%S]
            for fo in range(FO):
                pss = {t: ppool.tile([128, BC], f32, name=f"ps1_{c}_{fo}_{t}",
                                     tag="mm") for t in rows}
                for k in range(KX):
                    w_ap = w1x_sb[k][:, fo * 128:(fo + 1) * 128]
                    for t in rows:
                        nc.tensor.matmul(
                            pss[t][:], w_ap, C[(t * FO + k + 1) % S][:],
                            start=(k == 0), stop=(k == KX - 1))
                b1ap = bias1_sb[:, c * FO + fo:c * FO + fo + 1]
                for t in rows:
                    j = t * FO + fo
                    if t % 2 == 0:
                        nc.scalar.activation(A[j][:], pss[t][:], AF.Relu,
                                             bias=b1ap)
                    else:
                        # relu(psum + bias) on DVE: (psum add bias) max 0
                        nc.vector.tensor_scalar(A[j][:], pss[t][:], b1ap, 0.0,
                                                ADD, mybir.AluOpType.max)

        for gi, grp in enumerate(groups):
            pss = {}
            for t in grp:
                for fo in range(FO):
                    pss[(t, fo)] = ppool.tile([128, BC], f32,
                                              name=f"ps_p1_{t}_{fo}", tag="mm")
            pt = {}
            if gi == 0:
                # interleave w0p with row 0's tiles on scalar so the
                # k-th matmul's pair (w0p[k], prim(0,k)) lands together
                for k in range(KP):
                    nc.scalar.dma_start(w0p_sb[k][:],
                                        w0p_d[k * 128:(k + 1) * 128, :])
                    pt[(0, k)] = prim_dma(0, k)
            else:
                for k in range(KP):
                    for t in grp:
                        pt[(t, k)] = prim_dma(t, k)
            for k in range(KP):
                for fo in range(FO):
                    for t in grp:
                        nc.tensor.matmul(
                            pss[(t, fo)][:],
                            w0p_sb[k][:, fo * 128:(fo + 1) * 128],
                            pt[(t, k)][:],
                            start=(k == 0), stop=(k == KP - 1))
            load_deferred_consts(gi)
            for t in grp:
                for fo in range(FO):
                    j = t * FO + fo
                    nc.vector.tensor_copy(P[j][:], pss[(t, fo)][:])
                    ct = cpool.tile([128, BC], bf16, name=f"c0_{j}", tag="C")
                    nc.scalar.activation(ct[:], pss[(t, fo)][:], AF.Relu,
                                         bias=bias0_sb[:, fo:fo + 1])
                    C[j] = ct
        # col-0 layer1 with full run-of-8 weight reuse (bf16 phase-1 DMA
        # leaves enough bandwidth slack that no absorber work is needed)
        layer1_chunk(0, tuple(range(T)))

        # ==== layer emitters: (fo, k) outer, t inner -> weight run-of-8 ====
        def layer0_col(c):
            # C[4t+fo] = relu(W0x.T x + P + b0c);  x k-tile = A[(4t+k-1)%S]
            # The t sweep starts at t=1: the k=0 input A[4t-1] is a fo3
            # tile of the previous col's layer1, and t=0 needs A[31] --
            # the very LAST drain of that col.  Rotating gives each A
            # one extra sweep-step of drain slack.
            rows = [(1 + i) % T for i in range(T)]
            for fo in range(FO):
                pss = {t: ppool.tile([128, BC], f32, name=f"ps0_{c}_{fo}_{t}",
                                     tag="mm") for t in rows}
                for k in range(KX):
                    w_ap = w0x_sb[k][:, fo * 128:(fo + 1) * 128]
                    for t in rows:
                        nc.tensor.matmul(
                            pss[t][:], w_ap, A[(t * FO + k - 1) % S][:],
                            start=(k == 0), stop=(k == KX - 1))
                b0ap = bias0_sb[:, c * FO + fo:c * FO + fo + 1]
                for t in rows:
                    j = t * FO + fo
                    ct = cpool.tile([128, BC], bf16, name=f"c{c}_{j}", tag="C")
                    # ct = (psum + bias0_c) + P  on DVE, then relu on ACT
                    nc.vector.scalar_tensor_tensor(
                        ct[:], pss[t][:], b0ap, P[j][:], ADD, ADD)
                    nc.scalar.activation(ct[:], ct[:], AF.Relu)
                    C[j] = ct

        def layer1_col(c):
            # A[4t+fo] = relu(W1x.T C + b1c);  C k-tile = C[(4t+k+1)%S]
            for fo in range(FO):
                pss = [ppool.tile([128, BC], f32, name=f"ps1_{c}_{fo}_{t}",
                                  tag="mm") for t in range(T)]
                for k in range(KX):
                    w_ap = w1x_sb[k][:, fo * 128:(fo + 1) * 128]
                    for t in range(T):
                        nc.tensor.matmul(
                            pss[t][:], w_ap, C[(t * FO + k + 1) % S][:],
                            start=(k == 0), stop=(k == KX - 1))
                b1ap = bias1_sb[:, c * FO + fo:c * FO + fo + 1]
                for t in range(T):
                    j = t * FO + fo
                    if t % 2 == 0:
                        nc.scalar.activation(A[j][:], pss[t][:], AF.Relu,
                                             bias=b1ap)
                    else:
                        # relu(psum + bias) on DVE: (psum add bias) max 0
                        nc.vector.tensor_scalar(A[j][:], pss[t][:], b1ap, 0.0,
                                                ADD, mybir.AluOpType.max)

        # ==== cols 1..3 (col-0 layer1 was interleaved into phase 1) ====
        for c in range(1, NW):
            layer0_col(c)
            layer1_col(c)

        # ---- final: out = prev @ W_out + b_out;  prev[k] = A[(k-1) % S] ----
        psf_full = ppool.tile([128, BC], f32, name="psf", tag="mm")
        psf = psf_full[0:N_OUT, :]
        # emit in col-3's A-write order (sweep fo, then t) so the
        # accumulation chain chases the layer1 drains
        n = 0
        for fo in range(FO):
            for t in range(T):
                k = (t * FO + fo + 1) % S
                nc.tensor.matmul(
                    psf[:],
                    wout_sb[:, k * N_OUT:(k + 1) * N_OUT],
                    A[(k - 1) % S][:],
                    start=(n == 0), stop=(n == S - 1))
                n += 1
        out_sb = cpool.tile([N_OUT, BC], f32, name="out_sb", tag="C")
        nc.scalar.activation(out_sb[:], psf[:], AF.Identity, bias=bout_sb[:])
        nc.sync.dma_start(out_d[:, :], out_sb[:])

    nc.compile()

    names = dict(prim="prim_t", w0p="w0p", w0x="w0x", w1x="w1x",
                 wout="wout_packed", bias0="bias0", bias1="bias1",
                 bout="bout", out="out")
    _CACHE["nc"] = nc
    _CACHE["names"] = names
    return nc, names


def _make_in_maps(primary_input, W0, b0, W1, b1, W_out, b_out):
    """Host-side sharding + layout prep (all cheap numpy except the
    feature-major transpose of the batch shards)."""
    primary_input = np.ascontiguousarray(primary_input, dtype=np.float32)
    W0 = np.asarray(W0, dtype=np.float32)
    b0 = np.asarray(b0, dtype=np.float32)
    W1 = np.asarray(W1, dtype=np.float32)
    b1 = np.asarray(b1, dtype=np.float32)
    W_out = np.asarray(W_out, dtype=np.float32)
    b_out = np.asarray(b_out, dtype=np.float32)

    import ml_dtypes
    ps = D_IN // T  # 1024
    w0p = np.ascontiguousarray(W0[:ps].astype(ml_dtypes.bfloat16))  # [1024, 512]
    w0x = np.ascontiguousarray(W0[ps:ps + F].astype(ml_dtypes.bfloat16))
    w0_last = W0[ps + F]                             # [512]
    w1x = np.ascontiguousarray(W1[:F].astype(ml_dtypes.bfloat16))
    w1_last = W1[F]                                  # [512]

    bias0 = np.concatenate(
        [(b0 + c * w0_last).reshape(FO, 128).T for c in range(NW)], axis=1)
    bias1 = np.concatenate(
        [(b1 + c * w1_last).reshape(FO, 128).T for c in range(NW)], axis=1)
    bias0 = np.ascontiguousarray(bias0, dtype=np.float32)   # [128, 16]
    bias1 = np.ascontiguousarray(bias1, dtype=np.float32)   # [128, 16]

    # wout_packed[p, k*10+o] = W_out[128k+p, o]
    wout_packed = np.ascontiguousarray(
        W_out.reshape(S, 128, N_OUT).transpose(1, 0, 2).reshape(128, S * N_OUT)
        .astype(ml_dtypes.bfloat16))
    bout = np.ascontiguousarray(b_out.reshape(N_OUT, 1))

    shared = dict(w0p=w0p, w0x=w0x, w1x=w1x, wout_packed=wout_packed,
                  bias0=bias0, bias1=bias1, bout=bout)
    in_maps = []
    for core in range(N_CORES):
        shard = primary_input[core * BC:(core + 1) * BC]          # [512, 8192]
        prim_t = np.ascontiguousarray(shard.T.astype(ml_dtypes.bfloat16))
        m = {"prim_t": prim_t}
        m.update(shared)
        in_maps.append(m)
    return in_maps


def _install_ntff_hook():
    """Provide antenv.axon_hooks (absent in this image) backed by ctypes
    calls into libaxon_pjrt.so, so run_bass_kernel_spmd(trace=True) can
    capture NTFF profiles. Mirrors trn_agent_boot.trn_boot."""
    import contextlib
    import ctypes
    import sys
    import types

    if "antenv.axon_hooks" in sys.modules:
        return
    so_path = "/opt/axon/libaxon_pjrt.so"
    lib = ctypes.CDLL(so_path)
    lib.axon_start_nrt_profile.argtypes = [ctypes.POINTER(ctypes.c_int64),
                                           ctypes.c_size_t]
    lib.axon_start_nrt_profile.restype = ctypes.c_int64
    lib.axon_stop_nrt_profile.argtypes = [ctypes.c_char_p]
    lib.axon_stop_nrt_profile.restype = ctypes.c_int64

    @contextlib.contextmanager
    def _hook(output_dir, device_ids):
        import jax
        jax.devices()
        if device_ids:
            ids = (ctypes.c_int64 * len(device_ids))(*device_ids)
            rc = lib.axon_start_nrt_profile(ids, len(device_ids))
        else:
            rc = lib.axon_start_nrt_profile(None, 0)
        if rc != 0:
            raise RuntimeError(f"axon_start_nrt_profile rc={rc}")
        try:
            yield
        finally:
            n = lib.axon_stop_nrt_profile(str(output_dir).encode())
            print(f"profile: {n} file(s) written to {output_dir}",
                  file=sys.stderr)

    mod = types.ModuleType("antenv.axon_hooks")
    mod.get_axon_ntff_profile_hook = lambda: _hook
    mod.set_axon_ntff_profile_hook = lambda h: None
    sys.modules["antenv.axon_hooks"] = mod
    import antenv
    antenv.axon_hooks = mod


def kernel(primary_input, W0, b0, W1, b1, W_out, b_out, _trace=False,
           _trace_cores=None):
    from concourse import bass_utils

    if _trace:
        _install_ntff_hook()

    nc, _ = _build_program()
    in_maps = _make_in_maps(primary_input, W0, b0, W1, b1, W_out, b_out)
    res = bass_utils.run_bass_kernel_spmd(
        nc, in_maps, core_ids=list(range(N_CORES)),
        trace=_trace, trace_cores=_trace_cores)
    out = np.empty((B_FULL, N_OUT), dtype=np.float32)
    for core in range(N_CORES):
        out[core * BC:(core + 1) * BC] = res.results[core]["out"].T
    if _trace:
        kernel._last_results = res
    return out



# revision 18
# speedup vs baseline: 1.8690x; 1.0062x over previous
"""Capsule-network kernel for 8x TRN2 NeuronCores (data-parallel over batch).

Reference computation (see problem):
  prim = primary_input.reshape(B, 8, 1024)
  prev = zeros(B, 4096)
  for col in 0..3:
    # layer0: inp = [prim_t, x_t, col] (1537) @ W0 -> relu -> flat -> roll(-128)
    # layer1: inp = [x_t, col] (513) @ W1 -> relu -> flat -> roll(+128)
  out = prev @ W_out + b_out

Kernel strategy (per core, batch shard Bc=512):
  - Everything on-chip is FEATURE-MAJOR: tiles are [128 features, Bc batch].
    ROLL=128 == partition count, so rolls are free tile re-indexings.
  - The scalar `col` concat input contributes col*W[last_row] to the
    pre-activation -> folded into per-col biases (computed on host).
  - P = prim @ W0[0:1024] is col-invariant -> computed once (phase 1),
    kept in SBUF as bf16, added during the layer0 drain each col.
  - col 0 layer0 has x=0 -> out = relu(P + b0): no matmuls at all.
  - Matmuls run as bf16 (same 1 col/cycle PE rate as fp32r, but
    the 2-byte LDWEIGHTS hides fully: measured cadence ~216ns vs
    ~227ns for fp32r).  Activations/weights bf16, psum fp32.
  - HW measurement: an fp32r matmul whose stationary weights differ from
    the previous matmul costs ~252ns; same-weights runs cost ~226.7ns.
    So layers are swept (fo, k) outer / row t inner: 8 consecutive
    matmuls share one weight tile (one sweep = 8 psum banks).
  - ~16 dummy matmuls at t=0 (on a memset tile) ramp the PE out of its
    low p-state during the initial DMA wait.
"""

import numpy as np

# ---- problem constants (hardcoded; kernel.py must be self-contained) ----
B_FULL = 4096
D_IN = 8192
T = 8            # NUM_TALL
NW = 4           # NUM_WIDE
F = 512          # feature size per capsule row
ROLL = 128
N_CORES = 8
BC = B_FULL // N_CORES   # per-core batch = 512
S = (F * T) // 128       # state feature tiles = 32
KP = (D_IN // T) // 128  # prim k-tiles per capsule row = 8
KX = F // 128            # x k-tiles = 4
FO = F // 128            # output feature tiles per row-layer = 4
N_OUT = 10
N_WARM = 6               # dummy p-state warmup matmuls

_CACHE = {}


def _build_program():
    """Build (and cache) the single-core Bass program. Same program runs
    SPMD on all 8 cores with different batch shards."""
    if "nc" in _CACHE:
        return _CACHE["nc"], _CACHE["names"]

    from contextlib import ExitStack

    import concourse.tile as tile
    from concourse import bacc, mybir

    f32 = mybir.dt.float32
    f32r = mybir.dt.float32r
    bf16 = mybir.dt.bfloat16
    AF = mybir.ActivationFunctionType
    ADD = mybir.AluOpType.add

    nc = bacc.Bacc("TRN2", target_bir_lowering=False, debug=False,
                   num_devices=N_CORES)

    # prim + w0p travel and multiply as bf16: phase-1 is the only
    # DMA-heavy span (16.8MB of prim in fp32 saturates the 360 GB/s DMA
    # system during the cold start); halving the bytes costs ~26ns/matmul
    # of bf16 LDWEIGHTS overhead on the 256 P-matmuls but removes all
    # DMA-starvation stalls.
    prim_d = nc.dram_tensor("prim_t", [D_IN, BC], bf16, kind="ExternalInput").ap()
    w0p_d = nc.dram_tensor("w0p", [KP * 128, F], bf16, kind="ExternalInput").ap()
    w0x_d = nc.dram_tensor("w0x", [F, F], bf16, kind="ExternalInput").ap()
    w1x_d = nc.dram_tensor("w1x", [F, F], bf16, kind="ExternalInput").ap()
    wout_d = nc.dram_tensor("wout_packed", [128, S * N_OUT], bf16,
                            kind="ExternalInput").ap()
    bias0_d = nc.dram_tensor("bias0", [128, NW * FO], f32, kind="ExternalInput").ap()
    bias1_d = nc.dram_tensor("bias1", [128, NW * FO], f32, kind="ExternalInput").ap()
    bout_d = nc.dram_tensor("bout", [N_OUT, 1], f32, kind="ExternalInput").ap()
    out_d = nc.dram_tensor("out", [N_OUT, BC], f32, kind="ExternalOutput").ap()

    with tile.TileContext(nc) as tc, ExitStack() as ctx:
        const = ctx.enter_context(tc.tile_pool(name="const", bufs=1))
        state = ctx.enter_context(tc.tile_pool(name="state", bufs=1))
        cpool = ctx.enter_context(tc.tile_pool(name="cpool", bufs=33))
        prim_pool = ctx.enter_context(tc.tile_pool(name="primp", bufs=12))
        ppool = ctx.enter_context(tc.tile_pool(name="psum", bufs=8, space="PSUM"))

        # ---- constants ----
        w0p_sb = [const.tile([128, F], bf16, name=f"w0p{k}", tag=f"w0p{k}")
                  for k in range(KP)]
        w0x_sb = [const.tile([128, F], bf16, name=f"w0x{k}", tag=f"w0x{k}")
                  for k in range(KX)]
        w1x_sb = [const.tile([128, F], bf16, name=f"w1x{k}", tag=f"w1x{k}")
                  for k in range(KX)]
        wout_sb = const.tile([128, S * N_OUT], bf16, name="wout_sb", tag="wout")
        bias0_sb = const.tile([128, NW * FO], f32, name="bias0_sb", tag="bias0")
        bias1_sb = const.tile([128, NW * FO], f32, name="bias1_sb", tag="bias1")
        bout_sb = const.tile([N_OUT, 1], f32, name="bout_sb", tag="bout")
        warm_sb = const.tile([128, 128], f32, name="warm_sb", tag="warm")

        # ---- persistent state ----
        A = [state.tile([128, BC], bf16, name=f"state_a{i}", tag=f"A{i}")
             for i in range(S)]
        P = [state.tile([128, BC], bf16, name=f"state_p{i}", tag=f"P{i}")
             for i in range(S)]

        # ---- p-state warmup: dummy matmuls on a memset tile ----
        # (fp32 runs at 4 cycles/row so a 128-wide moving dim gives
        # ~213-790ns per dummy across the ramp)
        nc.vector.memset(warm_sb[:], 0.0)
        for i in range(N_WARM):
            ps = ppool.tile([128, BC], f32, name=f"warm{i}", tag="mm")
            nc.tensor.matmul(ps[0:128, 0:128], warm_sb[:], warm_sb[:],
                             start=True, stop=True)

        def load_deferred_consts(gi):
            # late-needed constants ride the idle gpsimd (SWDGE) queue so
            # the sync/scalar queues keep streaming prim.  w1x must be
            # resident by ~15us (first col0-layer1 chunk), so it goes out
            # at gi=0 -- the gpsimd queue has nothing else and issues it
            # at t~1us.
            if gi == 0:
                nc.sync.dma_start(bias0_sb[:], bias0_d[:, :])
                nc.gpsimd.dma_start(bias1_sb[:], bias1_d[:, :])
                nc.gpsimd.dma_start(bout_sb[:], bout_d[:, :])
                for k in range(KX):
                    nc.gpsimd.dma_start(w1x_sb[k][:], w1x_d[k * 128:(k + 1) * 128, :])
            elif gi == 1:
                for k in range(KX):
                    nc.gpsimd.dma_start(w0x_sb[k][:], w0x_d[k * 128:(k + 1) * 128, :])
            elif gi == 5:
                nc.gpsimd.dma_start(wout_sb[:], wout_d[:, :])

        C = [None] * S  # col-current layer0 outputs (cpool ring tiles)

        # ==== phase 1: P[j] = prim @ W0p (col-invariant), fused col0-L0 ====
        # Rows grouped [0], (1,2), (3,4), (5,6), [7]: within a group the
        # (k, fo) weight tile is reused across rows (run-of-2).
        # Drains: P copy (DVE, bf16) + col0-L0 C = relu(P+b0) (ACT).
        groups = [(t,) for t in range(T)]

        def prim_dma(t, k):
            # one k-tile per DMA, full-width: the DMA system is
            # descriptor-line-rate limited, so 1KB lines (full bf16 rows)
            # move twice the bytes per line vs split halves.  Row 0 and
            # odd rows ride the scalar queue (it starts issuing ~5us
            # before sync, which carries the TileContext preamble).
            g = t * KP + k
            tile_ = prim_pool.tile([128, BC], bf16, name=f"prim_{g}",
                                   tag="prim")
            # scalar (ACT engine: also runs drains) only carries row 0,
            # interleaved with w0p.  Sync takes rows 1/2/4/6 (row 1 first,
            # right after bias0, so it lands before its ~14us deadline);
            # gpsimd takes rows 3/5/7 after the small deferred consts.
            if t == 0:
                q = nc.scalar
            elif t in (1, 2, 4, 6):
                q = nc.sync
            else:
                q = nc.gpsimd
            q.dma_start(tile_[:], prim_d[g * 128:(g + 1) * 128, :])
            return tile_

        def layer1_chunk(c, rows):
            # layer1 for a subset of rows (weight run-of-len(rows)).
            # A[4t+fo] = relu(W1x.T C + b1c);  C k-tile = C[(4t+k+1)%S]
            for fo in range(FO):
                pss = {t: ppool.tile([128, BC], f32, name=f"ps1_{c}_{fo}_{t}",
                                     tag="mm") for t in rows}
                for k in range(KX):
                    w_ap = w1x_sb[k][:, fo * 128:(fo + 1) * 128]
                    for t in rows:
                        nc.tensor.matmul(
                            pss[t][:], w_ap, C[(t * FO + k + 1) % S][:],
                            start=(k == 0), stop=(k == KX - 1))
                b1ap = bias1_sb[:, c * FO + fo:c * FO + fo + 1]
                for t in rows:
                    j = t * FO + fo
                    if t % 2 == 0:
                        nc.scalar.activation(A[j][:], pss[t][:], AF.Relu,
                                             bias=b1ap)
                    else:
                        # relu(psum + bias) on DVE: (psum add bias) max 0
                        nc.vector.tensor_scalar(A[j][:], pss[t][:], b1ap, 0.0,
                                                ADD, mybir.AluOpType.max)

        for gi, grp in enumerate(groups):
            pss = {}
            for t in grp:
                for fo in range(FO):
                    pss[(t, fo)] = ppool.tile([128, BC], f32,
                                              name=f"ps_p1_{t}_{fo}", tag="mm")
            pt = {}
            if gi == 0:
                # interleave w0p with row 0's tiles on scalar so the
                # k-th matmul's pair (w0p[k], prim(0,k)) lands together
                for k in range(KP):
                    nc.scalar.dma_start(w0p_sb[k][:],
                                        w0p_d[k * 128:(k + 1) * 128, :])
                    pt[(0, k)] = prim_dma(0, k)
            else:
                for k in range(KP):
                    for t in grp:
                        pt[(t, k)] = prim_dma(t, k)
            for k in range(KP):
                for fo in range(FO):
                    for t in grp:
                        nc.tensor.matmul(
                            pss[(t, fo)][:],
                            w0p_sb[k][:, fo * 128:(fo + 1) * 128],
                            pt[(t, k)][:],
                            start=(k == 0), stop=(k == KP - 1))
            load_deferred_consts(gi)
            for t in grp:
                for fo in range(FO):
                    j = t * FO + fo
                    nc.vector.tensor_copy(P[j][:], pss[(t, fo)][:])
                    ct = cpool.tile([128, BC], bf16, name=f"c0_{j}", tag="C")
                    nc.scalar.activation(ct[:], pss[(t, fo)][:], AF.Relu,
                                         bias=bias0_sb[:, fo:fo + 1])
                    C[j] = ct
        # col-0 layer1 with full run-of-8 weight reuse (bf16 phase-1 DMA
        # leaves enough bandwidth slack that no absorber work is needed)
        layer1_chunk(0, tuple(range(T)))

        # ==== layer emitters: (fo, k) outer, t inner -> weight run-of-8 ====
        def layer0_col(c):
            # C[4t+fo] = relu(W0x.T x + P + b0c);  x k-tile = A[(4t+k-1)%S]
            # The t sweep starts at t=1: the k=0 input A[4t-1] is a fo3
            # tile of the previous col's layer1, and t=0 needs A[31] --
            # the very LAST drain of that col.  Rotating gives each A
            # one extra sweep-step of drain slack.
            rows = [(1 + i) % T for i in range(T)]
            for fo in range(FO):
                pss = {t: ppool.tile([128, BC], f32, name=f"ps0_{c}_{fo}_{t}",
                                     tag="mm") for t in rows}
                for k in range(KX):
                    w_ap = w0x_sb[k][:, fo * 128:(fo + 1) * 128]
                    for t in rows:
                        nc.tensor.matmul(
                            pss[t][:], w_ap, A[(t * FO + k - 1) % S][:],
                            start=(k == 0), stop=(k == KX - 1))
                b0ap = bias0_sb[:, c * FO + fo:c * FO + fo + 1]
                for t in rows:
                    j = t * FO + fo
                    ct = cpool.tile([128, BC], bf16, name=f"c{c}_{j}", tag="C")
                    # ct = (psum + bias0_c) + P  on DVE, then relu on ACT
                    nc.vector.scalar_tensor_tensor(
                        ct[:], pss[t][:], b0ap, P[j][:], ADD, ADD)
                    nc.scalar.activation(ct[:], ct[:], AF.Relu)
                    C[j] = ct

        def layer1_col(c):
            # A[4t+fo] = relu(W1x.T C + b1c);  C k-tile = C[(4t+k+1)%S]
            for fo in range(FO):
                pss = [ppool.tile([128, BC], f32, name=f"ps1_{c}_{fo}_{t}",
                                  tag="mm") for t in range(T)]
                for k in range(KX):
                    w_ap = w1x_sb[k][:, fo * 128:(fo + 1) * 128]
                    for t in range(T):
                        nc.tensor.matmul(
                            pss[t][:], w_ap, C[(t * FO + k + 1) % S][:],
                            start=(k == 0), stop=(k == KX - 1))
                b1ap = bias1_sb[:, c * FO + fo:c * FO + fo + 1]
                for t in range(T):
                    j = t * FO + fo
                    if t % 2 == 0:
                        nc.scalar.activation(A[j][:], pss[t][:], AF.Relu,
                                             bias=b1ap)
                    else:
                        # relu(psum + bias) on DVE: (psum add bias) max 0
                        nc.vector.tensor_scalar(A[j][:], pss[t][:], b1ap, 0.0,
                                                ADD, mybir.AluOpType.max)

        # ==== cols 1..3 (col-0 layer1 was interleaved into phase 1) ====
        for c in range(1, NW):
            layer0_col(c)
            layer1_col(c)

        # ---- final: out = prev @ W_out + b_out;  prev[k] = A[(k-1) % S] ----
        psf_full = ppool.tile([128, BC], f32, name="psf", tag="mm")
        psf = psf_full[0:N_OUT, :]
        # emit in col-3's A-write order (sweep fo, then t) so the
        # accumulation chain chases the layer1 drains
        n = 0
        for fo in range(FO):
            for t in range(T):
                k = (t * FO + fo + 1) % S
                nc.tensor.matmul(
                    psf[:],
                    wout_sb[:, k * N_OUT:(k + 1) * N_OUT],
                    A[(k - 1) % S][:],
                    start=(n == 0), stop=(n == S - 1))
                n += 1
        out_sb = cpool.tile([N_OUT, BC], f32, name="out_sb", tag="C")
        nc.scalar.activation(out_sb[:], psf[:], AF.Identity, bias=bout_sb[:])
        nc.sync.dma_start(out_d[:, :], out_sb[:])

    nc.compile()

    names = dict(prim="prim_t", w0p="w0p", w0x="w0x", w1x="w1x",
                 wout="wout_packed", bias0="bias0", bias1="bias1",
                 bout="bout", out="out")
    _CACHE["nc"] = nc
    _CACHE["names"] = names
    return nc, names


def _make_in_maps(primary_input, W0, b0, W1, b1, W_out, b_out):
    """Host-side sharding + layout prep (all cheap numpy except the
    feature-major transpose of the batch shards)."""
    primary_input = np.ascontiguousarray(primary_input, dtype=np.float32)
    W0 = np.asarray(W0, dtype=np.float32)
    b0 = np.asarray(b0, dtype=np.float32)
    W1 = np.asarray(W1, dtype=np.float32)
    b1 = np.asarray(b1, dtype=np.float32)
    W_out = np.asarray(W_out, dtype=np.float32)
    b_out = np.asarray(b_out, dtype=np.float32)

    import ml_dtypes
    ps = D_IN // T  # 1024
    w0p = np.ascontiguousarray(W0[:ps].astype(ml_dtypes.bfloat16))  # [1024, 512]
    w0x = np.ascontiguousarray(W0[ps:ps + F].astype(ml_dtypes.bfloat16))
    w0_last = W0[ps + F]                             # [512]
    w1x = np.ascontiguousarray(W1[:F].astype(ml_dtypes.bfloat16))
    w1_last = W1[F]                                  # [512]

    bias0 = np.concatenate(
        [(b0 + c * w0_last).reshape(FO, 128).T for c in range(NW)], axis=1)
    bias1 = np.concatenate(
        [(b1 + c * w1_last).reshape(FO, 128).T for c in range(NW)], axis=1)
    bias0 = np.ascontiguousarray(bias0, dtype=np.float32)   # [128, 16]
    bias1 = np.ascontiguousarray(bias1, dtype=np.float32)   # [128, 16]

    # wout_packed[p, k*10+o] = W_out[128k+p, o]
    wout_packed = np.ascontiguousarray(
        W_out.reshape(S, 128, N_OUT).transpose(1, 0, 2).reshape(128, S * N_OUT)
        .astype(ml_dtypes.bfloat16))
    bout = np.ascontiguousarray(b_out.reshape(N_OUT, 1))

    shared = dict(w0p=w0p, w0x=w0x, w1x=w1x, wout_packed=wout_packed,
                  bias0=bias0, bias1=bias1, bout=bout)
    in_maps = []
    for core in range(N_CORES):
        shard = primary_input[core * BC:(core + 1) * BC]          # [512, 8192]
        prim_t = np.ascontiguousarray(shard.T.astype(ml_dtypes.bfloat16))
        m = {"prim_t": prim_t}
        m.update(shared)
        in_maps.append(m)
    return in_maps


def _install_ntff_hook():
    """Provide antenv.axon_hooks (absent in this image) backed by ctypes
    calls into libaxon_pjrt.so, so run_bass_kernel_spmd(trace=True) can
    capture NTFF profiles. Mirrors trn_agent_boot.trn_boot."""
    import contextlib
    import ctypes
    import sys
    import types

    if "antenv.axon_hooks" in sys.modules:
        return
    so_path = "/opt/axon/libaxon_pjrt.so"
    lib = ctypes.CDLL(so_path)
    lib.axon_start_nrt_profile.argtypes = [ctypes.POINTER(ctypes.c_int64),
                                           ctypes.c_size_t]
    lib.axon_start_nrt_profile.restype = ctypes.c_int64
    lib.axon_stop_nrt_profile.argtypes = [ctypes.c_char_p]
    lib.axon_stop_nrt_profile.restype = ctypes.c_int64

    @contextlib.contextmanager
    def _hook(output_dir, device_ids):
        import jax
        jax.devices()
        if device_ids:
            ids = (ctypes.c_int64 * len(device_ids))(*device_ids)
            rc = lib.axon_start_nrt_profile(ids, len(device_ids))
        else:
            rc = lib.axon_start_nrt_profile(None, 0)
        if rc != 0:
            raise RuntimeError(f"axon_start_nrt_profile rc={rc}")
        try:
            yield
        finally:
            n = lib.axon_stop_nrt_profile(str(output_dir).encode())
            print(f"profile: {n} file(s) written to {output_dir}",
                  file=sys.stderr)

    mod = types.ModuleType("antenv.axon_hooks")
    mod.get_axon_ntff_profile_hook = lambda: _hook
    mod.set_axon_ntff_profile_hook = lambda h: None
    sys.modules["antenv.axon_hooks"] = mod
    import antenv
    antenv.axon_hooks = mod


def kernel(primary_input, W0, b0, W1, b1, W_out, b_out, _trace=False,
           _trace_cores=None):
    from concourse import bass_utils

    if _trace:
        _install_ntff_hook()

    nc, _ = _build_program()
    in_maps = _make_in_maps(primary_input, W0, b0, W1, b1, W_out, b_out)
    res = bass_utils.run_bass_kernel_spmd(
        nc, in_maps, core_ids=list(range(N_CORES)),
        trace=_trace, trace_cores=_trace_cores)
    out = np.empty((B_FULL, N_OUT), dtype=np.float32)
    for core in range(N_CORES):
        out[core * BC:(core + 1) * BC] = res.results[core]["out"].T
    if _trace:
        kernel._last_results = res
    return out

